# revision 1
# baseline (speedup 1.0000x reference)
"""Single-head attention (B=4, S=2048, E=1024, fp32) on 8 trn2 NeuronCores.

Sharding: (batch, q-half) -> 8 shards. Core c handles batch c//2, query rows
[h*1024, (h+1)*1024) with h = c%2. Each core computes K/V projections for the
full 2048-row sequence of its batch (duplicated within the pair), its own Q
half, scores^T, softmax (no max subtraction -- scores are O(1) here), and the
output rows.

Device kernel layouts (per core):
  xt  [E, S]   x[b].T with the core's q-half columns permuted first
               (softmax/output are invariant to key order, so K/V may use the
               permuted order as long as it is consistent).
  QT  [f, q]   f on partitions -> scores contraction over f needs this.
  KT  [f, s]   same.
  S^T [k, q]   k on partitions -> rowsum via matmul with ones, O uses P^T
               directly as the stationary operand.
  V   [s, f]   natural layout, moving operand of the O matmul.

P^T = exp(S^T) is bounced through DRAM ([k_tile, q_tile, 128, 128] tiles) so
SBUF pool lifetimes nest: {xt,qt,kt} die before {wvt,v} are allocated.

All matmuls run as float32r (full fp32 data, 1 cycle/row on the PE for moving
dim >= 256).
"""

import numpy as np

P = 128


def _emit(nc, E=1024, S=2048, SQ=1024, SB=256):
    """Emit the per-core kernel IR into `nc`."""
    import concourse.mybir as mybir
    import concourse.tile as tile

    f32 = mybir.dt.float32
    f32r = mybir.dt.float32r
    ACT = mybir.ActivationFunctionType

    ET = E // P          # e/f tiles (8)
    ST = S // P          # s/k tiles (16)
    STH = ST // 2        # k tiles per half (8)
    QTN = SQ // P        # q tiles (8)
    NQC = SQ // 512      # q chunks of 512 (2)
    NFC = E // 512       # f chunks of 512 (2)
    NSB = SB // P        # s-subtiles per V stationary block (2)

    xt = nc.dram_tensor("xt", [E, S], f32r, kind="ExternalInput")
    xv = nc.dram_tensor("xv", [S // SB, ET, P, SB], f32r, kind="ExternalInput")
    wq4 = nc.dram_tensor("wq4", [ET, P, ET, P], f32r, kind="ExternalInput")  # [f,p,e,c]
    wk4 = nc.dram_tensor("wk4", [ET, P, ET, P], f32r, kind="ExternalInput")  # [f,p,e,c]
    wvt = nc.dram_tensor("wvt", [E, E], f32r, kind="ExternalInput")
    bq8 = nc.dram_tensor("bq8", [P, ET], f32, kind="ExternalInput")
    bk8 = nc.dram_tensor("bk8", [P, ET], f32, kind="ExternalInput")
    bvb = nc.dram_tensor("bvb", [P, E], f32, kind="ExternalInput")
    ones2 = nc.dram_tensor("ones2", [P, 2], f32r, kind="ExternalInput")
    id2 = nc.dram_tensor("id2", [2, 2], f32, kind="ExternalInput")
    o = nc.dram_tensor("o", [SQ, E], f32, kind="ExternalOutput")

    with tile.TileContext(nc) as tc:
        dram_cm = tc.tile_pool(name="dramp", bufs=1, space="DRAM")
        dramp = dram_cm.__enter__()
        ptda = dramp.tile([STH, P, SQ], f32r, tag="ptda")
        ptdb = dramp.tile([STH, P, SQ], f32r, tag="ptdb")
        psum_cm = tc.tile_pool(name="psum", bufs=4, space="PSUM")
        psum = psum_cm.__enter__()
        small_cm = tc.tile_pool(name="small", bufs=1)
        small = small_cm.__enter__()

        # qt + second kt half live until the end of phase 2
        qk_cm = tc.tile_pool(name="qk", bufs=1)
        qk = qk_cm.__enter__()
        qt_t = qk.tile([P, ET, SQ], f32r, tag="qt")
        kt_b = qk.tile([P, ET, S // 2], f32r, tag="ktb")
        exp_cm = tc.tile_pool(name="expp", bufs=5)
        expp = exp_cm.__enter__()
        # first kt half in its own pool: released mid-phase-2 so the V-phase
        # inputs (wvt, xv) can start loading while scores still run
        kta_cm = tc.tile_pool(name="kta", bufs=1)
        ktap = kta_cm.__enter__()
        kt_a = ktap.tile([P, ET, S // 2], f32r, tag="kta")

        bq_t = small.tile([P, ET], f32, tag="bq")
        nc.sync.dma_start(bq_t[:], bq8[:])
        bk_t = small.tile([P, ET], f32, tag="bk")
        nc.sync.dma_start(bk_t[:], bk8[:])
        bv_t = small.tile([P, E], f32, tag="bv")
        nc.sync.dma_start(bv_t[:], bvb[:])
        ones_t = small.tile([P, 2], f32r, tag="ones")
        nc.sync.dma_start(ones_t[:], ones2[:])
        id2_t = small.tile([2, 2], f32, tag="id2")
        nc.sync.dma_start(id2_t[:], id2[:])
        rs_sb = small.tile([2, SQ], f32, tag="rssb")

        # ---------------- phase 1: QT and KT projections ----------------
        xt_cm = tc.tile_pool(name="xtp", bufs=1)
        xtp = xt_cm.__enter__()
        w_cm = tc.tile_pool(name="wstream", bufs=3)
        wsp = w_cm.__enter__()

        xt_t = xtp.tile([P, ET, S], f32r, tag="xt")
        # first Q weight row, then xt by s-chunk (all e of a chunk together) so
        # the first accumulation group is ready after ~2.5MB instead of ~8.5MB
        def xt_chunk_dma(j):
            for e in range(ET):
                nc.sync.dma_start(
                    xt_t[:, e, j * 512 : (j + 1) * 512],
                    xt[e * P : (e + 1) * P, j * 512 : (j + 1) * 512],
                )

        wq_rows = []
        w_t = wsp.tile([P, ET, P], f32r, tag="w", name="wq_f0")
        nc.sync.dma_start(w_t[:], wq4[0])
        wq_rows.append(w_t)
        # chunk 0 lands in 256-wide halves so the first (split) accumulation
        # group only waits on ~1.5MB
        for half in range(2):
            for e in range(ET):
                nc.sync.dma_start(
                    xt_t[:, e, half * 256 : (half + 1) * 256],
                    xt[e * P : (e + 1) * P, half * 256 : (half + 1) * 256],
                )
        for j in range(1, SQ // 512):  # remaining chunks Q needs
            xt_chunk_dma(j)
        for f in range(1, ET):
            w_t = wsp.tile([P, ET, P], f32r, tag="w", name=f"wq_f{f}")
            nc.sync.dma_start(w_t[:], wq4[f])
            wq_rows.append(w_t)
        wk_rows = []
        w_t = wsp.tile([P, ET, P], f32r, tag="wk", name="wk_f0")
        nc.sync.dma_start(w_t[:], wk4[0])
        wk_rows.append(w_t)
        for j in range(SQ // 512, S // 512):  # remaining chunks for K
            xt_chunk_dma(j)
        for f in range(1, ET):
            w_t = wsp.tile([P, ET, P], f32r, tag="wk", name=f"wk_f{f}")
            nc.sync.dma_start(w_t[:], wk4[f])
            wk_rows.append(w_t)

        def kt_slice(j512):
            # j-th 512-wide chunk of the K output, routed to the right half
            half, jj = divmod(j512, (S // 2) // 512)
            t = (kt_a, kt_b)[half]
            return t, jj

        # first Q group split into two 256-wide halves for an earlier start
        for half in range(2):
            pst = psum.tile([P, 512], f32, tag="mm", name=f"psh{half}")
            for e in range(ET):
                nc.tensor.matmul(
                    pst[:, :256],
                    wq_rows[0][:, e],
                    xt_t[:, e, half * 256 : (half + 1) * 256],
                    start=(e == 0),
                    stop=(e == ET - 1),
                )
            nc.scalar.add(
                qt_t[:, 0, half * 256 : (half + 1) * 256],
                pst[:, :256],
                bq_t[:, 0:1],
            )

        for proj, (w_rows, bias_t, ncols) in enumerate(
            ((wq_rows, bq_t, SQ), (wk_rows, bk_t, S))
        ):
            ncc = ncols // 512
            for f in range(ET):
                w_t = w_rows[f]
                for j in range(ncc):
                    if proj == 0 and f == 0 and j == 0:
                        continue  # handled by the split groups above
                    pst = psum.tile([P, 512], f32, tag="mm", name=f"ps{j}")
                    for e in range(ET):
                        nc.tensor.matmul(
                            pst[:],
                            w_t[:, e],
                            xt_t[:, e, j * 512 : (j + 1) * 512],
                            start=(e == 0),
                            stop=(e == ET - 1),
                        )
                    if proj == 0:
                        out_ap = qt_t[:, f, j * 512 : (j + 1) * 512]
                    else:
                        t, jj = kt_slice(j)
                        out_ap = t[:, f, jj * 512 : (jj + 1) * 512]
                    nc.scalar.add(out_ap, pst[:], bias_t[:, f : f + 1])
        w_cm.__exit__(None, None, None)
        xt_cm.__exit__(None, None, None)

        # ---------------- phase 2: scores^T + exp -> PT (to DRAM) ----------------
        rs_ps = [
            psum.tile([2, 512], f32, tag=f"rsacc{qc}", name=f"rsacc{qc}", bufs=1)
            for qc in range(NQC)
        ]

        def scores_ktile(k):
            kt_t = kt_a if k < STH else kt_b
            kk = k % STH
            ps = [
                psum.tile([P, 512], f32, tag="mm", name=f"ps{j}")
                for j in range(NQC)
            ]
            for f in range(ET):
                for qc in range(NQC):
                    nc.tensor.matmul(
                        ps[qc][:],
                        kt_t[:, f, kk * P : (kk + 1) * P],
                        qt_t[:, f, qc * 512 : (qc + 1) * 512],
                        start=(f == 0),
                        stop=(f == ET - 1),
                    )
            for qc in range(NQC):
                e_t = expp.tile([P, 512], f32r, tag="exp")
                nc.scalar.activation(e_t[:], ps[qc][:], ACT.Exp)
                ptdh = ptda if k < STH else ptdb
                nc.gpsimd.dma_start(
                    ptdh[k % STH, :, qc * 512 : (qc + 1) * 512],
                    e_t[:],
                )
                # rowsum over this k-tile: ones^T @ exp -> [2, 512]
                nc.tensor.matmul(
                    rs_ps[qc][:],
                    ones_t[:],
                    e_t[:],
                    start=(k == 0),
                    stop=(k == ST - 1),
                )

        for k in range(STH):
            scores_ktile(k)
        kta_cm.__exit__(None, None, None)
        for k in range(STH, ST):
            scores_ktile(k)
        for qc in range(NQC):
            nc.vector.tensor_copy(
                rs_sb[:, qc * 512 : (qc + 1) * 512], rs_ps[qc][:]
            )

        # ---------------- phase 3: V projection ----------------
        v_cm = tc.tile_pool(name="vp", bufs=1)
        vp = v_cm.__enter__()
        v_halves = [
            vp.tile([P, STH, E], f32r, tag=f"v{h}", name=f"v{h}") for h in range(2)
        ]
        wv_cm = tc.tile_pool(name="wvp", bufs=1)
        wvp = wv_cm.__enter__()
        wvt_t = wvp.tile([P, ET, E], f32r, tag="wvt")
        for e in range(ET):
            nc.scalar.dma_start(wvt_t[:, e], wvt[e * P : (e + 1) * P, :])
        xs_cm = tc.tile_pool(name="xstream", bufs=3)
        xsp = xs_cm.__enter__()

        for sb in range(S // SB):
            xv_t = xsp.tile([P, ET, SB], f32r, tag="xv")
            nc.scalar.dma_start(xv_t[:], xv[sb].rearrange("e p c -> p e c"))
            for si in range(NSB):
                ps = [
                    psum.tile([P, 512], f32, tag="mm", name=f"ps{fc}")
                    for fc in range(NFC)
                ]
                for e in range(ET):
                    for fc in range(NFC):
                        nc.tensor.matmul(
                            ps[fc][:],
                            xv_t[:, e, si * P : (si + 1) * P],
                            wvt_t[:, e, fc * 512 : (fc + 1) * 512],
                            start=(e == 0),
                            stop=(e == ET - 1),
                        )
                st = sb * NSB + si
                vh = v_halves[st // STH]
                for fc in range(NFC):
                    nc.vector.tensor_add(
                        vh[:, st % STH, fc * 512 : (fc + 1) * 512],
                        ps[fc][:],
                        bv_t[:, fc * 512 : (fc + 1) * 512],
                    )
        xs_cm.__exit__(None, None, None)
        wv_cm.__exit__(None, None, None)

        # ---------------- phase 4: O = softmax-normalized P^T.T @ V ----------------
        pts_cm = tc.tile_pool(name="pts", bufs=3)
        pts = pts_cm.__enter__()
        ob_cm = tc.tile_pool(name="ob", bufs=3)
        obp = ob_cm.__enter__()
        for qp in range(QTN // 2):
            pt_ts = []
            for h, ptdh in enumerate((ptda, ptdb)):
                pt_t = pts.tile([P, STH, 2 * P], f32r, tag=f"pt{h}", name=f"pt{h}")
                nc.sync.dma_start(
                    pt_t[:],
                    ptdh[:, :, qp * 2 * P : (qp + 1) * 2 * P].rearrange(
                        "k p q -> p k q"
                    ),
                )
                pt_ts.append(pt_t)
            for sub in range(2):
                qt_i = qp * 2 + sub
                po = [
                    psum.tile([P, 512], f32, tag="mm", name=f"po{j}")
                    for j in range(NFC)
                ]
                prs = psum.tile([P, 2], f32, tag="rs", bufs=2)
                nc.tensor.matmul(
                    prs[:],
                    rs_sb[:, qt_i * P : (qt_i + 1) * P],
                    id2_t[:],
                    is_transpose=True,
                )
                for k in range(ST):
                    lhs = pt_ts[k // STH][:, k % STH, sub * P : (sub + 1) * P]
                    vh = v_halves[k // STH]
                    for fc in range(NFC):
                        nc.tensor.matmul(
                            po[fc][:],
                            lhs,
                            vh[:, k % STH, fc * 512 : (fc + 1) * 512],
                            start=(k == 0),
                            stop=(k == ST - 1),
                        )
                recip = obp.tile([P, 1], f32, tag="recip")
                nc.vector.reciprocal(recip[:], prs[:, 0:1])
                o_t = obp.tile([P, E], f32, tag="ob")
                for fc in range(NFC):
                    nc.vector.tensor_scalar_mul(
                        o_t[:, fc * 512 : (fc + 1) * 512], po[fc][:], recip[:]
                    )
                    nc.sync.dma_start(
                        o[qt_i * P : (qt_i + 1) * P, fc * 512 : (fc + 1) * 512],
                        o_t[:, fc * 512 : (fc + 1) * 512],
                    )
        ob_cm.__exit__(None, None, None)
        pts_cm.__exit__(None, None, None)

        v_cm.__exit__(None, None, None)
        exp_cm.__exit__(None, None, None)
        qk_cm.__exit__(None, None, None)
        small_cm.__exit__(None, None, None)
        psum_cm.__exit__(None, None, None)
        dram_cm.__exit__(None, None, None)


_NC_CACHE = {}


def build_nc(E=1024, S=2048, SQ=1024, SB=256):
    key = (E, S, SQ, SB)
    if key in _NC_CACHE:
        return _NC_CACHE[key]
    import concourse.bacc as bacc

    nc = bacc.Bacc(None, target_bir_lowering=False)
    _emit(nc, E=E, S=S, SQ=SQ, SB=SB)
    nc.finalize()
    _NC_CACHE[key] = nc
    return nc


def _round_f32r(a):
    """Round fp32 to fp32r (tf32-like: 11 explicit mantissa bits, RNE)."""
    u = np.ascontiguousarray(a, np.float32).view(np.uint32)
    u = u + np.uint32(0x7FF) + ((u >> np.uint32(12)) & np.uint32(1))
    return (u & np.uint32(0xFFFFF000)).view(np.float32)


def make_in_maps(x, Wq, bq, Wk, bk, Wv, bv, E=1024, S=2048, SQ=1024, SB=256):
    """Host-side prep: per-core input dicts for run_bass_kernel_spmd."""
    ET = E // P
    scale = 1.0 / np.sqrt(np.float32(E))
    x = np.asarray(x, np.float32)
    B = x.shape[0]
    n_half = S // SQ

    # Weight tiles [e_tile, f_tile, p, f] so each stationary DMA is contiguous.
    def tile4(wt):  # wt: [E, E] (e rows, f cols) -> [f_tile, p(e), e_tile, c(f)]
        return np.ascontiguousarray(wt.reshape(ET, P, ET, P).transpose(2, 1, 0, 3))

    wq4 = _round_f32r(tile4(np.asarray(Wq, np.float32).T * scale))
    wk4 = _round_f32r(tile4(np.asarray(Wk, np.float32).T))
    wvt_h = _round_f32r(np.ascontiguousarray(np.asarray(Wv, np.float32).T))
    bq8 = np.ascontiguousarray((np.asarray(bq, np.float32) * scale).reshape(ET, P).T)
    bk8 = np.ascontiguousarray(np.asarray(bk, np.float32).reshape(ET, P).T)
    bvb = np.ascontiguousarray(np.broadcast_to(np.asarray(bv, np.float32), (P, E)))

    in_maps = []
    for c in range(B * n_half):
        b, h = divmod(c, n_half)
        xt_full = x[b].T  # [E, S]
        order = [h] + [i for i in range(n_half) if i != h]
        xt_perm = _round_f32r(
            np.concatenate([xt_full[:, i * SQ : (i + 1) * SQ] for i in order], axis=1)
        )
        xv = np.ascontiguousarray(
            xt_perm.reshape(ET, P, S // SB, SB).transpose(2, 0, 1, 3)
        )
        in_maps.append(
            {
                "ones2": np.ones((P, 2), np.float32),
                "id2": np.eye(2, dtype=np.float32),
                "xt": xt_perm,
                "xv": xv,
                "wq4": wq4,
                "wk4": wk4,
                "wvt": wvt_h,
                "bq8": bq8,
                "bk8": bk8,
                "bvb": bvb,
            }
        )
    return in_maps


def kernel(x, Wq, bq, Wk, bk, Wv, bv):
    from concourse.bass_utils import run_bass_kernel_spmd

    E, S, SQ = 1024, 2048, 1024
    x = np.asarray(x, np.float32)
    B = x.shape[0]
    nc = build_nc(E=E, S=S, SQ=SQ)
    in_maps = make_in_maps(x, Wq, bq, Wk, bk, Wv, bv, E=E, S=S, SQ=SQ)
    n_cores = len(in_maps)
    res = run_bass_kernel_spmd(nc, in_maps, list(range(n_cores)))
    out = np.empty((B, S, E), np.float32)
    n_half = S // SQ
    for c in range(n_cores):
        b, h = divmod(c, n_half)
        out[b, h * SQ : (h + 1) * SQ, :] = res.results[c]["o"]
    return out



# revision 3
# speedup vs baseline: 1.0077x; 1.0077x over previous
"""Single-head attention (B=4, S=2048, E=1024, fp32) on 8 trn2 NeuronCores.

Sharding: (batch, key-half) -> 8 shards. Core c handles batch c//2 and the
key/value rows [h*1024, (h+1)*1024) with h = c%2. Each core computes the Q
projection for ALL 2048 queries of its batch, K/V projections for its own
1024 keys, exp(scores^T) against those keys, the unnormalized partial output
O_h = exp(S^T)^T @ V_h and the partial softmax denominators rs_h. The host
combines: out = (O_0 + O_1) / (rs_0 + rs_1) + bv  (the V bias commutes with
the softmax average, so it is added once on the host).

All matmul operands are bf16 (rel-err budget is 2e-2; bf16 keeps us ~2e-3),
which halves SBUF/DMA footprint so everything stays resident in SBUF:

  xt [128, 8e, 2048]  x[b]^T, this core's key-half columns permuted first.
  w  [128, 2, 8f, 8e, 128]  Wq^T*scale and Wk^T stationary tiles.
  qt [128, 8f, 2048]  Q^T (f on partitions) - moving operand of scores.
  kt [128, 8f, 1024]  K^T - stationary of scores.
  wv [128, 8e, 1024]  Wv^T - moving operand of the V projection.
  v  [128, 8k, 1024]  V (k on partitions) - moving operand of O.
  pt [128, 8k, 2048]  exp(S^T) (k on partitions) - stationary of O.

Rowsums come from ones^T @ exp tiles on the PE. A burst of tiny warmup
matmuls runs during the initial input DMA so the PE's activity-based clock
ramp (1.2 -> 2.4 GHz) completes before the first real matmul.
"""

import numpy as np

P = 128


def _emit(nc, E=1024, S=2048, SK=1024):
    import concourse.mybir as mybir
    import concourse.tile as tile

    f32 = mybir.dt.float32
    bf16 = mybir.dt.bfloat16
    ACT = mybir.ActivationFunctionType

    ET = E // P     # e/f tiles (8)
    QT = S // P     # q tiles (16)
    KT = SK // P    # k tiles (8)
    NQC = S // 512  # q chunks (4)
    NKC = SK // 512  # k chunks (2)
    NFC = E // 512  # f chunks (2)

    xt8 = nc.dram_tensor("xt8", [ET, P, S], bf16, kind="ExternalInput")
    wq8 = nc.dram_tensor("wq8", [ET, P, ET, P], bf16, kind="ExternalInput")
    wk8 = nc.dram_tensor("wk8", [ET, P, ET, P], bf16, kind="ExternalInput")
    wv8 = nc.dram_tensor("wv8", [ET, P, E], bf16, kind="ExternalInput")
    bq8 = nc.dram_tensor("bq8", [P, ET], f32, kind="ExternalInput")
    bk8 = nc.dram_tensor("bk8", [P, ET], f32, kind="ExternalInput")
    ones8 = nc.dram_tensor("ones8", [P, ET], bf16, kind="ExternalInput")
    ou = nc.dram_tensor("ou", [S, E], bf16, kind="ExternalOutput")
    rs = nc.dram_tensor("rs", [1, S], f32, kind="ExternalOutput")

    with tile.TileContext(nc) as tc:
        with (
            tc.tile_pool(name="psum", bufs=4, space="PSUM") as psum,
            tc.tile_pool(name="small", bufs=1) as small,
            tc.tile_pool(name="persist", bufs=1) as pers,
            tc.tile_pool(name="obuf", bufs=3) as obp,
        ):
            ones_t = small.tile([P, ET], bf16, tag="ones")
            nc.sync.dma_start(ones_t[:], ones8[:])
            bq_t = small.tile([P, ET], f32, tag="bq")
            nc.sync.dma_start(bq_t[:], bq8[:])
            bk_t = small.tile([P, ET], f32, tag="bk")
            nc.sync.dma_start(bk_t[:], bk8[:])
            rs_sb = small.tile([1, S], f32, tag="rssb")

            xt_t = pers.tile([P, ET, S], bf16, tag="xt")
            w_t = pers.tile([P, 2, ET, ET, P], bf16, tag="w")
            wv_t = pers.tile([P, ET, E], bf16, tag="wv")
            qt_t = pers.tile([P, ET, S], bf16, tag="qt")
            kt_t = pers.tile([P, ET, SK], bf16, tag="kt")
            v_t = pers.tile([P, KT, E], bf16, tag="v")
            pt_t = pers.tile([P, KT, S], bf16, tag="pt")

            # ---- input DMA: sync ring gets weights, gpsimd ring gets xt ----
            nc.sync.dma_start(w_t[:, 0, 0], wq8[0])
            for e in range(ET):  # first q-chunk of xt, by e-slice
                nc.gpsimd.dma_start(
                    xt_t[:, e, 0:512], xt8[e, :, 0:512]
                )
            for f in range(1, ET):
                nc.sync.dma_start(w_t[:, 0, f], wq8[f])
            for e in range(ET):  # rest of xt in one wide contiguous DMA each
                nc.gpsimd.dma_start(
                    xt_t[:, e, 512:S], xt8[e, :, 512:S]
                )
            for f in range(ET):
                nc.sync.dma_start(w_t[:, 1, f], wk8[f])
            for e in range(ET):
                nc.scalar.dma_start(wv_t[:, e], wv8[e])

            # ---- PE warmup (HAM clock ramp) while input DMAs stream ----
            warm_ps = psum.tile([1, 512], f32, tag="rs0", bufs=1)
            for i in range(48):
                nc.tensor.matmul(
                    warm_ps[:, 0:8], ones_t[:, 0:1], ones_t[:], start=True, stop=True
                )

            # ---- Q projection: qt[f, q] = sum_e w[e, f] * xt[e, q] ----
            for qc in range(NQC):
                for f in range(ET):
                    ps = psum.tile([P, 512], f32, tag="mm", name=f"q{qc}_{f}")
                    for e in range(ET):
                        nc.tensor.matmul(
                            ps[:],
                            w_t[:, 0, f, e],
                            xt_t[:, e, qc * 512 : (qc + 1) * 512],
                            start=(e == 0),
                            stop=(e == ET - 1),
                        )
                    nc.scalar.add(
                        qt_t[:, f, qc * 512 : (qc + 1) * 512], ps[:], bq_t[:, f : f + 1]
                    )

            # ---- K projection (key half = first SK columns of xt) ----
            for kc in range(NKC):
                for f in range(ET):
                    ps = psum.tile([P, 512], f32, tag="mm", name=f"k{kc}_{f}")
                    for e in range(ET):
                        nc.tensor.matmul(
                            ps[:],
                            w_t[:, 1, f, e],
                            xt_t[:, e, kc * 512 : (kc + 1) * 512],
                            start=(e == 0),
                            stop=(e == ET - 1),
                        )
                    nc.scalar.add(
                        kt_t[:, f, kc * 512 : (kc + 1) * 512], ps[:], bk_t[:, f : f + 1]
                    )

            # ---- V projection: v[k, f] = sum_e xt[e, k] * wv[e, f] (no bias) ----
            for kt in range(KT):
                ps2 = [
                    psum.tile([P, 512], f32, tag="mm", name=f"v{kt}_{fc}")
                    for fc in range(NFC)
                ]
                for e in range(ET):
                    for fc in range(NFC):
                        nc.tensor.matmul(
                            ps2[fc][:],
                            xt_t[:, e, kt * P : (kt + 1) * P],
                            wv_t[:, e, fc * 512 : (fc + 1) * 512],
                            start=(e == 0),
                            stop=(e == ET - 1),
                        )
                for fc in range(NFC):
                    nc.vector.tensor_copy(
                        v_t[:, kt, fc * 512 : (fc + 1) * 512], ps2[fc][:]
                    )

            # ---- scores^T + exp + rowsum, two passes over q halves ----
            for qh in range(2):
                rs_ps = [
                    psum.tile(
                        [1, 512], f32, tag=f"rs{qh * 2 + qc}",
                        name=f"rs{qh * 2 + qc}", bufs=1,
                    )
                    for qc in range(2)
                ]
                for kt in range(KT):
                    ps2 = [
                        psum.tile([P, 512], f32, tag="mm", name=f"s{kt}_{qc}")
                        for qc in range(2)
                    ]
                    for f in range(ET):
                        for qc in range(2):
                            nc.tensor.matmul(
                                ps2[qc][:],
                                kt_t[:, f, kt * P : (kt + 1) * P],
                                qt_t[:, f, qh * 1024 + qc * 512 : qh * 1024 + (qc + 1) * 512],
                                start=(f == 0),
                                stop=(f == ET - 1),
                            )
                    for qc in range(2):
                        col = qh * 1024 + qc * 512
                        nc.scalar.activation(
                            pt_t[:, kt, col : col + 512], ps2[qc][:], ACT.Exp
                        )
                        nc.tensor.matmul(
                            rs_ps[qc][:],
                            ones_t[:, 0:1],
                            pt_t[:, kt, col : col + 512],
                            start=(kt == 0),
                            stop=(kt == KT - 1),
                        )
                for qc in range(2):
                    col = qh * 1024 + qc * 512
                    nc.vector.tensor_copy(rs_sb[:, col : col + 512], rs_ps[qc][:])

            # ---- O = pt^T @ v, unnormalized; store bf16 ----
            for qt in range(QT):
                po = [
                    psum.tile([P, 512], f32, tag="mm", name=f"o{qt}_{fc}")
                    for fc in range(NFC)
                ]
                for kt in range(KT):
                    for fc in range(NFC):
                        nc.tensor.matmul(
                            po[fc][:],
                            pt_t[:, kt, qt * P : (qt + 1) * P],
                            v_t[:, kt, fc * 512 : (fc + 1) * 512],
                            start=(kt == 0),
                            stop=(kt == KT - 1),
                        )
                o_sb = obp.tile([P, E], bf16, tag="ob")
                for fc in range(NFC):
                    nc.vector.tensor_copy(
                        o_sb[:, fc * 512 : (fc + 1) * 512], po[fc][:]
                    )
                nc.gpsimd.dma_start(ou[qt * P : (qt + 1) * P, :], o_sb[:])
            nc.sync.dma_start(rs[:], rs_sb[:])


_NC_CACHE = {}


def build_nc(E=1024, S=2048, SK=1024):
    key = (E, S, SK)
    if key in _NC_CACHE:
        return _NC_CACHE[key]
    import concourse.bacc as bacc

    nc = bacc.Bacc(None, target_bir_lowering=False)
    _emit(nc, E=E, S=S, SK=SK)
    nc.finalize()
    _NC_CACHE[key] = nc
    return nc


def make_in_maps(x, Wq, bq, Wk, bk, Wv, bv, E=1024, S=2048, SK=1024):
    """Host-side prep: per-core input dicts for run_bass_kernel_spmd."""
    import ml_dtypes

    bf16 = ml_dtypes.bfloat16
    ET = E // P
    scale = np.float32(1.0 / np.sqrt(np.float32(E)))
    x = np.asarray(x, np.float32)
    B = x.shape[0]
    n_half = S // SK

    def wtile(w):  # [f,p?] -> [f_tile, p(e), e_tile, c(f)] stationary blocks
        return np.ascontiguousarray(
            np.asarray(w, np.float32).reshape(ET, P, ET, P).transpose(0, 3, 2, 1)
        ).astype(bf16)

    wq8 = wtile(np.asarray(Wq, np.float32) * scale)
    wk8 = wtile(Wk)
    # wv8[e, p, f] = Wv[f, e*128+p]
    wv8 = np.ascontiguousarray(
        np.asarray(Wv, np.float32).T.reshape(ET, P, E)
    ).astype(bf16)
    bq8 = np.ascontiguousarray((np.asarray(bq, np.float32) * scale).reshape(ET, P).T)
    bk8 = np.ascontiguousarray(np.asarray(bk, np.float32).reshape(ET, P).T)
    ones8 = np.ones((P, ET), bf16)

    in_maps = []
    for c in range(B * n_half):
        b, h = divmod(c, n_half)
        xt_full = x[b].T  # [E, S]
        if h == 1:
            xt_full = np.concatenate([xt_full[:, SK:], xt_full[:, :SK]], axis=1)
        xt8 = np.ascontiguousarray(xt_full.reshape(ET, P, S)).astype(bf16)
        in_maps.append(
            {
                "xt8": xt8,
                "wq8": wq8,
                "wk8": wk8,
                "wv8": wv8,
                "bq8": bq8,
                "bk8": bk8,
                "ones8": ones8,
            }
        )
    return in_maps


def kernel(x, Wq, bq, Wk, bk, Wv, bv):
    from concourse.bass_utils import run_bass_kernel_spmd

    E, S, SK = 1024, 2048, 1024
    x = np.asarray(x, np.float32)
    B = x.shape[0]
    n_half = S // SK
    nc = build_nc(E=E, S=S, SK=SK)
    in_maps = make_in_maps(x, Wq, bq, Wk, bk, Wv, bv, E=E, S=S, SK=SK)
    n_cores = len(in_maps)
    res = run_bass_kernel_spmd(nc, in_maps, list(range(n_cores)))

    bvf = np.asarray(bv, np.float32)
    out = np.empty((B, S, E), np.float32)
    for b in range(B):
        osum = None
        rsum = None
        for h in range(n_half):
            r = res.results[b * n_half + h]
            o_h = np.asarray(r["ou"]).astype(np.float32)
            rs_h = np.asarray(r["rs"]).astype(np.float32).reshape(S)
            if h == 1:  # undo the query permutation
                o_h = np.concatenate([o_h[SK:], o_h[:SK]], axis=0)
                rs_h = np.concatenate([rs_h[SK:], rs_h[:SK]])
            osum = o_h if osum is None else osum + o_h
            rsum = rs_h if rsum is None else rsum + rs_h
        out[b] = osum / rsum[:, None] + bvf[None, :]
    return out


# revision 5
# speedup vs baseline: 1.1922x; 1.1831x over previous
"""Single-head attention (B=4, S=2048, E=1024, fp32) on 8 trn2 NeuronCores.

Sharding: (batch, key-half) -> 8 shards. Core c handles batch c//2 and the
key/value rows [h*1024, (h+1)*1024) with h = c%2. Each core computes the Q
projection for ALL 2048 queries of its batch, K/V projections for its own
1024 keys, exp(scores^T) against those keys, the unnormalized partial output
O_h = exp(S^T)^T @ V_h and the partial softmax denominators rs_h. The host
combines: out = (O_0 + O_1) / (rs_0 + rs_1) + bv  (the V bias commutes with
the softmax average, so it is added once on the host).

Dtype split: STATIONARY matmul operands are fp32r (standard 2-XBUS
LDWEIGHTS hides behind the moving stream; bf16 FWL grabs all 4 XBUSes and
serializes ~45ns/matmul), MOVING operands are bf16 (halves SBUF + DMA).
Exception: the V projection's stationary is the bf16 xt tile (cheaper than
keeping a second fp32r copy of x).

  xt [128, 8e, 2048] bf16  x[b]^T, key-half columns first (host permute).
  w  [128, 8e, 128] f32r   Wq^T*scale / Wk^T stationary tiles (streamed).
  qt [128, 8f, 2048] bf16  Q^T - moving operand of scores.
  kt [128, 8f, 1024] f32r  K^T - stationary of scores.
  wv [128, 8e, 1024] bf16  Wv^T - moving operand of the V projection.
  v  [128, 8k, 1024] bf16  V - moving operand of O.
  pt [128, 8k, 2048] f32r  exp(S^T) - stationary of O, moving of rowsum.

Rowsums come from ones^T @ exp tiles on the PE. A burst of tiny warmup
matmuls runs during the initial input DMA so the PE's activity-based clock
ramp (1.2 -> 2.4 GHz) completes before the first real matmul.
"""

import numpy as np

P = 128


def _emit(nc, E=1024, S=2048, SK=1024):
    import concourse.mybir as mybir
    import concourse.tile as tile

    f32 = mybir.dt.float32
    f32r = mybir.dt.float32r
    bf16 = mybir.dt.bfloat16
    ACT = mybir.ActivationFunctionType

    ET = E // P     # e/f tiles (8)
    QT = S // P     # q tiles (16)
    KT = SK // P    # k tiles (8)
    NQC = S // 512  # q chunks (4)
    NKC = SK // 512  # k chunks (2)
    NFC = E // 512  # f chunks (2)

    xt8 = nc.dram_tensor("xt8", [ET, P, S], bf16, kind="ExternalInput")
    wq8 = nc.dram_tensor("wq8", [ET, P, ET, P], bf16, kind="ExternalInput")
    wk8 = nc.dram_tensor("wk8", [ET, P, ET, P], bf16, kind="ExternalInput")
    wv8 = nc.dram_tensor("wv8", [ET, P, E], bf16, kind="ExternalInput")
    bq8 = nc.dram_tensor("bq8", [P, ET], f32, kind="ExternalInput")
    bk8 = nc.dram_tensor("bk8", [P, ET], f32, kind="ExternalInput")
    ones8 = nc.dram_tensor("ones8", [P, ET], bf16, kind="ExternalInput")
    ou = nc.dram_tensor("ou", [S, E], bf16, kind="ExternalOutput")
    rs = nc.dram_tensor("rs", [1, S], f32, kind="ExternalOutput")

    with tile.TileContext(nc) as tc:
        with (
            tc.tile_pool(name="psum", bufs=6, space="PSUM") as psum,
            tc.tile_pool(name="small", bufs=1) as small,
            tc.tile_pool(name="persist", bufs=1) as pers,
            tc.tile_pool(name="obuf", bufs=3) as obp,
        ):
            ones_t = small.tile([P, ET], bf16, tag="ones")
            nc.sync.dma_start(ones_t[:], ones8[:])
            bq_t = small.tile([P, ET], f32, tag="bq")
            nc.sync.dma_start(bq_t[:], bq8[:])
            bk_t = small.tile([P, ET], f32, tag="bk")
            nc.sync.dma_start(bk_t[:], bk8[:])
            rs_sb = small.tile([1, S], f32, tag="rssb")

            qt_t = pers.tile([P, ET, S], bf16, tag="qt")
            kt_t = pers.tile([P, ET, SK], bf16, tag="kt")
            v_t = pers.tile([P, KT, E], bf16, tag="v")

            rs_ps = [
                psum.tile([1, 512], f32, tag=f"rs{qc}", name=f"rs{qc}", bufs=1)
                for qc in range(2)
            ]

            with (
                tc.tile_pool(name="ph1", bufs=1) as ph1,
                tc.tile_pool(name="wstream", bufs=3) as wsp,
            ):
                xt_t = ph1.tile([P, ET, S], bf16, tag="xt")
                wv_t = ph1.tile([P, ET, E], bf16, tag="wv")

                # ---- input DMA: sync ring = weights, gpsimd ring = xt ----
                wk_rows = []
                w_t = wsp.tile([P, ET, P], bf16, tag="w", name="wk_f0")
                nc.sync.dma_start(w_t[:], wk8[0])
                wk_rows.append(w_t)
                for e in range(ET):  # first 512 columns of xt, by e-slice
                    nc.gpsimd.dma_start(xt_t[:, e, 0:512], xt8[e, :, 0:512])
                for f in range(1, ET):
                    w_t = wsp.tile([P, ET, P], bf16, tag="w", name=f"wk_f{f}")
                    nc.sync.dma_start(w_t[:], wk8[f])
                    wk_rows.append(w_t)
                for e in range(ET):  # rest of xt, one wide contiguous DMA each
                    nc.gpsimd.dma_start(xt_t[:, e, 512:S], xt8[e, :, 512:S])
                wq_rows = []
                for f in range(ET):
                    w_t = wsp.tile([P, ET, P], bf16, tag="w", name=f"wq_f{f}")
                    nc.sync.dma_start(w_t[:], wq8[f])
                    wq_rows.append(w_t)
                for e in range(ET):
                    nc.scalar.dma_start(wv_t[:, e], wv8[e])

                # ---- PE warmup (HAM clock ramp) while input DMAs stream ----
                for i in range(48):
                    nc.tensor.matmul(
                        rs_ps[0][:, 0:8], ones_t[:, 0:1], ones_t[:],
                        start=True, stop=True,
                    )

                # ---- K projection (key half = first SK columns of xt) ----
                for f in range(ET):
                    ps2 = [
                        psum.tile([P, 512], f32, tag="mm", name=f"k{f}_{kc}")
                        for kc in range(NKC)
                    ]
                    for e in range(ET):
                        for kc in range(NKC):
                            nc.tensor.matmul(
                                ps2[kc][:],
                                wk_rows[f][:, e],
                                xt_t[:, e, kc * 512 : (kc + 1) * 512],
                                start=(e == 0),
                                stop=(e == ET - 1),
                            )
                    for kc in range(NKC):
                        nc.scalar.add(
                            kt_t[:, f, kc * 512 : (kc + 1) * 512],
                            ps2[kc][:],
                            bk_t[:, f : f + 1],
                        )

                # ---- Q projection over all queries ----
                for f in range(ET):
                    ps4 = [
                        psum.tile([P, 512], f32, tag="mm", name=f"q{f}_{qc}")
                        for qc in range(NQC)
                    ]
                    for e in range(ET):
                        for qc in range(NQC):
                            nc.tensor.matmul(
                                ps4[qc][:],
                                wq_rows[f][:, e],
                                xt_t[:, e, qc * 512 : (qc + 1) * 512],
                                start=(e == 0),
                                stop=(e == ET - 1),
                            )
                    for qc in range(NQC):
                        nc.scalar.add(
                            qt_t[:, f, qc * 512 : (qc + 1) * 512],
                            ps4[qc][:],
                            bq_t[:, f : f + 1],
                        )

                # ---- V projection: v[k, f] = sum_e xt[e, k] * wv[e, f] ----
                for kt in range(KT):
                    ps2 = [
                        psum.tile([P, 512], f32, tag="mm", name=f"v{kt}_{fc}")
                        for fc in range(NFC)
                    ]
                    for e in range(ET):
                        for fc in range(NFC):
                            nc.tensor.matmul(
                                ps2[fc][:],
                                xt_t[:, e, kt * P : (kt + 1) * P],
                                wv_t[:, e, fc * 512 : (fc + 1) * 512],
                                start=(e == 0),
                                stop=(e == ET - 1),
                            )
                    for fc in range(NFC):
                        nc.vector.tensor_copy(
                            v_t[:, kt, fc * 512 : (fc + 1) * 512], ps2[fc][:]
                        )

            with tc.tile_pool(name="ptp", bufs=1) as ptp:
                pt_t = ptp.tile([P, KT, S], bf16, tag="pt")

                # ---- scores^T + exp + rowsum, two passes over q halves ----
                for qh in range(2):
                    for kt in range(KT):
                        ps2 = [
                            psum.tile([P, 512], f32, tag="mm", name=f"s{kt}_{qc}")
                            for qc in range(2)
                        ]
                        for f in range(ET):
                            for qc in range(2):
                                col = qh * 1024 + qc * 512
                                nc.tensor.matmul(
                                    ps2[qc][:],
                                    kt_t[:, f, kt * P : (kt + 1) * P],
                                    qt_t[:, f, col : col + 512],
                                    start=(f == 0),
                                    stop=(f == ET - 1),
                                )
                        for qc in range(2):
                            col = qh * 1024 + qc * 512
                            nc.scalar.activation(
                                pt_t[:, kt, col : col + 512], ps2[qc][:], ACT.Exp
                            )
                            nc.tensor.matmul(
                                rs_ps[qc][:],
                                ones_t[:, 0:1],
                                pt_t[:, kt, col : col + 512],
                                start=(kt == 0),
                                stop=(kt == KT - 1),
                            )
                    for qc in range(2):
                        col = qh * 1024 + qc * 512
                        nc.vector.tensor_copy(
                            rs_sb[:, col : col + 512], rs_ps[qc][:]
                        )

                # ---- O = pt^T @ v, unnormalized; store bf16 ----
                for qt in range(QT):
                    po = [
                        psum.tile([P, 512], f32, tag="mm", name=f"o{qt}_{fc}")
                        for fc in range(NFC)
                    ]
                    for kt in range(KT):
                        for fc in range(NFC):
                            nc.tensor.matmul(
                                po[fc][:],
                                pt_t[:, kt, qt * P : (qt + 1) * P],
                                v_t[:, kt, fc * 512 : (fc + 1) * 512],
                                start=(kt == 0),
                                stop=(kt == KT - 1),
                            )
                    o_sb = obp.tile([P, E], bf16, tag="ob")
                    for fc in range(NFC):
                        nc.vector.tensor_copy(
                            o_sb[:, fc * 512 : (fc + 1) * 512], po[fc][:]
                        )
                    nc.gpsimd.dma_start(ou[qt * P : (qt + 1) * P, :], o_sb[:])
                nc.sync.dma_start(rs[:], rs_sb[:])


_NC_CACHE = {}


def build_nc(E=1024, S=2048, SK=1024):
    key = (E, S, SK)
    if key in _NC_CACHE:
        return _NC_CACHE[key]
    import concourse.bacc as bacc

    nc = bacc.Bacc(None, target_bir_lowering=False)
    _emit(nc, E=E, S=S, SK=SK)
    nc.finalize()
    _NC_CACHE[key] = nc
    return nc


def _round_f32r(a):
    """Round fp32 to fp32r (tf32-like: 11 explicit mantissa bits, RNE)."""
    u = np.ascontiguousarray(a, np.float32).view(np.uint32)
    u = u + np.uint32(0x7FF) + ((u >> np.uint32(12)) & np.uint32(1))
    return (u & np.uint32(0xFFFFF000)).view(np.float32)


def make_in_maps(x, Wq, bq, Wk, bk, Wv, bv, E=1024, S=2048, SK=1024):
    """Host-side prep: per-core input dicts for run_bass_kernel_spmd."""
    import ml_dtypes

    bf16 = ml_dtypes.bfloat16
    ET = E // P
    scale = np.float32(1.0 / np.sqrt(np.float32(E)))
    x = np.asarray(x, np.float32)
    B = x.shape[0]
    n_half = S // SK

    def wtile(w):  # [f_tile, p(e), e_tile, c(f)] stationary blocks
        return np.ascontiguousarray(
            np.asarray(w, np.float32).reshape(ET, P, ET, P).transpose(0, 3, 2, 1)
        ).astype(bf16)

    wq8 = wtile(np.asarray(Wq, np.float32) * scale)
    wk8 = wtile(Wk)
    # wv8[e, p, f] = Wv[f, e*128+p]
    wv8 = np.ascontiguousarray(
        np.asarray(Wv, np.float32).T.reshape(ET, P, E)
    ).astype(bf16)
    bq8 = np.ascontiguousarray((np.asarray(bq, np.float32) * scale).reshape(ET, P).T)
    bk8 = np.ascontiguousarray(np.asarray(bk, np.float32).reshape(ET, P).T)
    ones8 = np.ones((P, ET), bf16)

    in_maps = []
    for c in range(B * n_half):
        b, h = divmod(c, n_half)
        xt_full = x[b].T  # [E, S]
        if h == 1:
            xt_full = np.concatenate([xt_full[:, SK:], xt_full[:, :SK]], axis=1)
        xt8 = np.ascontiguousarray(xt_full.reshape(ET, P, S)).astype(bf16)
        in_maps.append(
            {
                "xt8": xt8,
                "wq8": wq8,
                "wk8": wk8,
                "wv8": wv8,
                "bq8": bq8,
                "bk8": bk8,
                "ones8": ones8,
            }
        )
    return in_maps


def kernel(x, Wq, bq, Wk, bk, Wv, bv):
    from concourse.bass_utils import run_bass_kernel_spmd

    E, S, SK = 1024, 2048, 1024
    x = np.asarray(x, np.float32)
    B = x.shape[0]
    n_half = S // SK
    nc = build_nc(E=E, S=S, SK=SK)
    in_maps = make_in_maps(x, Wq, bq, Wk, bk, Wv, bv, E=E, S=S, SK=SK)
    n_cores = len(in_maps)
    res = run_bass_kernel_spmd(nc, in_maps, list(range(n_cores)))

    bvf = np.asarray(bv, np.float32)
    out = np.empty((B, S, E), np.float32)
    for b in range(B):
        osum = None
        rsum = None
        for h in range(n_half):
            r = res.results[b * n_half + h]
            o_h = np.asarray(r["ou"]).astype(np.float32)
            rs_h = np.asarray(r["rs"]).astype(np.float32).reshape(S)
            if h == 1:  # undo the query permutation
                o_h = np.concatenate([o_h[SK:], o_h[:SK]], axis=0)
                rs_h = np.concatenate([rs_h[SK:], rs_h[:SK]])
            osum = o_h if osum is None else osum + o_h
            rsum = rs_h if rsum is None else rsum + rs_h
        out[b] = osum / rsum[:, None] + bvf[None, :]
    return out


# revision 7
# speedup vs baseline: 1.2157x; 1.0197x over previous
"""Single-head attention (B=4, S=2048, E=1024, fp32) on 8 trn2 NeuronCores.

Sharding: (batch, key-half) -> 8 shards. Core c handles batch c//2 and the
key/value rows [h*1024, (h+1)*1024) with h = c%2. Each core computes the Q
projection for ALL 2048 queries of its batch, K/V projections for its own
1024 keys, exp(scores^T) against those keys, the unnormalized partial output
O_h = exp(S^T)^T @ V_h and the partial softmax denominators rs_h. The host
combines: out = (O_0 + O_1) / (rs_0 + rs_1) + bv  (the V bias commutes with
the softmax average, so it is added once on the host).

Dtype split: STATIONARY matmul operands are fp32r (standard 2-XBUS
LDWEIGHTS hides behind the moving stream; bf16 FWL grabs all 4 XBUSes and
serializes ~45ns/matmul), MOVING operands are bf16 (halves SBUF + DMA).
Exception: the V projection's stationary is the bf16 xt tile (cheaper than
keeping a second fp32r copy of x).

  xt [128, 8e, 2048] bf16  x[b]^T, key-half columns first (host permute).
  w  [128, 8e, 128] f32r   Wq^T*scale / Wk^T stationary tiles (streamed).
  qt [128, 8f, 2048] bf16  Q^T - moving operand of scores.
  kt [128, 8f, 1024] f32r  K^T - stationary of scores.
  wv [128, 8e, 1024] bf16  Wv^T - moving operand of the V projection.
  v  [128, 8k, 1024] bf16  V - moving operand of O.
  pt [128, 8k, 2048] f32r  exp(S^T) - stationary of O, moving of rowsum.

Rowsums come from ones^T @ exp tiles on the PE. A burst of tiny warmup
matmuls runs during the initial input DMA so the PE's activity-based clock
ramp (1.2 -> 2.4 GHz) completes before the first real matmul.
"""

import numpy as np

P = 128


def _emit(nc, E=1024, S=2048, SK=1024):
    import concourse.mybir as mybir
    import concourse.tile as tile

    f32 = mybir.dt.float32
    f32r = mybir.dt.float32r
    bf16 = mybir.dt.bfloat16
    ACT = mybir.ActivationFunctionType

    ET = E // P     # e/f tiles (8)
    QT = S // P     # q tiles (16)
    KT = SK // P    # k tiles (8)
    NQC = S // 512  # q chunks (4)
    NKC = SK // 512  # k chunks (2)
    NFC = E // 512  # f chunks (2)

    xt8 = nc.dram_tensor("xt8", [ET, P, S], bf16, kind="ExternalInput")
    wq8 = nc.dram_tensor("wq8", [ET, P, ET, P], bf16, kind="ExternalInput")
    wk8 = nc.dram_tensor("wk8", [ET, P, ET, P], bf16, kind="ExternalInput")
    wv8 = nc.dram_tensor("wv8", [ET, P, E], bf16, kind="ExternalInput")
    bq8 = nc.dram_tensor("bq8", [P, ET], f32, kind="ExternalInput")
    bk8 = nc.dram_tensor("bk8", [P, ET], f32, kind="ExternalInput")
    ones8 = nc.dram_tensor("ones8", [P, ET], bf16, kind="ExternalInput")
    ou = nc.dram_tensor("ou", [S, E], bf16, kind="ExternalOutput")
    rs = nc.dram_tensor("rs", [1, S], f32, kind="ExternalOutput")

    with tile.TileContext(nc) as tc:
        with (
            tc.tile_pool(name="psum", bufs=6, space="PSUM") as psum,
            tc.tile_pool(name="small", bufs=1) as small,
            tc.tile_pool(name="persist", bufs=1) as pers,
            tc.tile_pool(name="obuf", bufs=3) as obp,
        ):
            ones_t = small.tile([P, ET], bf16, tag="ones")
            nc.sync.dma_start(ones_t[:], ones8[:])
            bq_t = small.tile([P, ET], f32, tag="bq")
            nc.sync.dma_start(bq_t[:], bq8[:])
            bk_t = small.tile([P, ET], f32, tag="bk")
            nc.sync.dma_start(bk_t[:], bk8[:])
            rs_sb = small.tile([1, S], f32, tag="rssb")

            qt_t = pers.tile([P, ET, S], bf16, tag="qt")
            kt_t = pers.tile([P, ET, SK], bf16, tag="kt")
            v_t = pers.tile([P, KT, E], bf16, tag="v")

            rs_ps = [
                psum.tile([1, 512], f32, tag=f"rs{qc}", name=f"rs{qc}", bufs=1)
                for qc in range(2)
            ]

            with (
                tc.tile_pool(name="ph1", bufs=1) as ph1,
                tc.tile_pool(name="wstream", bufs=3) as wsp,
            ):
                xt_t = ph1.tile([P, ET, S], bf16, tag="xt")
                wv_t = ph1.tile([P, ET, E], bf16, tag="wv")

                # ---- input DMA: sync ring = weights; xt alternates over the
                # gpsimd + vector rings, key-half columns first ----
                wk_rows = []
                w_t = wsp.tile([P, ET, P], bf16, tag="w", name="wk_f0")
                nc.sync.dma_start(w_t[:], wk8[0])
                wk_rows.append(w_t)
                xt_rings = [nc.gpsimd, nc.scalar]
                for kc in range(NKC):  # key half, chunk-sized for early starts
                    for e in range(ET):
                        xt_rings[e % 2].dma_start(
                            xt_t[:, e, kc * 512 : (kc + 1) * 512],
                            xt8[e, :, kc * 512 : (kc + 1) * 512],
                        )
                for f in range(1, ET):
                    w_t = wsp.tile([P, ET, P], bf16, tag="w", name=f"wk_f{f}")
                    nc.sync.dma_start(w_t[:], wk8[f])
                    wk_rows.append(w_t)
                for e in range(ET):  # q-only columns, one wide DMA each
                    xt_rings[e % 2].dma_start(xt_t[:, e, SK:S], xt8[e, :, SK:S])
                wq_rows = []
                for f in range(ET):
                    w_t = wsp.tile([P, ET, P], bf16, tag="w", name=f"wq_f{f}")
                    nc.sync.dma_start(w_t[:], wq8[f])
                    wq_rows.append(w_t)
                for e in range(ET):
                    nc.scalar.dma_start(wv_t[:, e], wv8[e])

                # ---- PE warmup (HAM clock ramp) while input DMAs stream ----
                for i in range(32):
                    nc.tensor.matmul(
                        rs_ps[0][:, 0:8], ones_t[:, 0:1], ones_t[:],
                        start=True, stop=True,
                    )

                # ---- K projection (key half = first SK columns of xt) ----
                for f in range(ET):
                    ps2 = [
                        psum.tile([P, 512], f32, tag="mm", name=f"k{f}_{kc}")
                        for kc in range(NKC)
                    ]
                    for e in range(ET):
                        for kc in range(NKC):
                            nc.tensor.matmul(
                                ps2[kc][:],
                                wk_rows[f][:, e],
                                xt_t[:, e, kc * 512 : (kc + 1) * 512],
                                start=(e == 0),
                                stop=(e == ET - 1),
                            )
                    for kc in range(NKC):
                        nc.scalar.add(
                            kt_t[:, f, kc * 512 : (kc + 1) * 512],
                            ps2[kc][:],
                            bk_t[:, f : f + 1],
                        )

                # ---- Q projection over all queries ----
                for f in range(ET):
                    ps4 = [
                        psum.tile([P, 512], f32, tag="mm", name=f"q{f}_{qc}")
                        for qc in range(NQC)
                    ]
                    for e in range(ET):
                        for qc in range(NQC):
                            nc.tensor.matmul(
                                ps4[qc][:],
                                wq_rows[f][:, e],
                                xt_t[:, e, qc * 512 : (qc + 1) * 512],
                                start=(e == 0),
                                stop=(e == ET - 1),
                            )
                    for qc in range(NQC):
                        nc.scalar.add(
                            qt_t[:, f, qc * 512 : (qc + 1) * 512],
                            ps4[qc][:],
                            bq_t[:, f : f + 1],
                        )

                # ---- V projection: v[k, f] = sum_e xt[e, k] * wv[e, f] ----
                for kt in range(KT):
                    ps2 = [
                        psum.tile([P, 512], f32, tag="mm", name=f"v{kt}_{fc}")
                        for fc in range(NFC)
                    ]
                    for e in range(ET):
                        for fc in range(NFC):
                            nc.tensor.matmul(
                                ps2[fc][:],
                                xt_t[:, e, kt * P : (kt + 1) * P],
                                wv_t[:, e, fc * 512 : (fc + 1) * 512],
                                start=(e == 0),
                                stop=(e == ET - 1),
                            )
                    for fc in range(NFC):
                        nc.vector.tensor_copy(
                            v_t[:, kt, fc * 512 : (fc + 1) * 512], ps2[fc][:]
                        )

            with tc.tile_pool(name="ptp", bufs=1) as ptp:
                pt_t = ptp.tile([P, KT, S], bf16, tag="pt")

                # ---- scores^T + exp + rowsum, two passes over q halves ----
                for qh in range(2):
                    for kt in range(KT):
                        ps2 = [
                            psum.tile([P, 512], f32, tag="mm", name=f"s{kt}_{qc}")
                            for qc in range(2)
                        ]
                        for f in range(ET):
                            for qc in range(2):
                                col = qh * 1024 + qc * 512
                                nc.tensor.matmul(
                                    ps2[qc][:],
                                    kt_t[:, f, kt * P : (kt + 1) * P],
                                    qt_t[:, f, col : col + 512],
                                    start=(f == 0),
                                    stop=(f == ET - 1),
                                )
                        for qc in range(2):
                            col = qh * 1024 + qc * 512
                            nc.scalar.activation(
                                pt_t[:, kt, col : col + 512], ps2[qc][:], ACT.Exp
                            )
                            nc.tensor.matmul(
                                rs_ps[qc][:],
                                ones_t[:, 0:1],
                                pt_t[:, kt, col : col + 512],
                                start=(kt == 0),
                                stop=(kt == KT - 1),
                            )
                    for qc in range(2):
                        col = qh * 1024 + qc * 512
                        nc.vector.tensor_copy(
                            rs_sb[:, col : col + 512], rs_ps[qc][:]
                        )

                # ---- O = pt^T @ v, unnormalized; store bf16 ----
                for qt in range(QT):
                    po = [
                        psum.tile([P, 512], f32, tag="mm", name=f"o{qt}_{fc}")
                        for fc in range(NFC)
                    ]
                    for kt in range(KT):
                        for fc in range(NFC):
                            nc.tensor.matmul(
                                po[fc][:],
                                pt_t[:, kt, qt * P : (qt + 1) * P],
                                v_t[:, kt, fc * 512 : (fc + 1) * 512],
                                start=(kt == 0),
                                stop=(kt == KT - 1),
                            )
                    o_sb = obp.tile([P, E], bf16, tag="ob")
                    for fc in range(NFC):
                        nc.vector.tensor_copy(
                            o_sb[:, fc * 512 : (fc + 1) * 512], po[fc][:]
                        )
                    nc.gpsimd.dma_start(ou[qt * P : (qt + 1) * P, :], o_sb[:])
                nc.sync.dma_start(rs[:], rs_sb[:])


_NC_CACHE = {}


def build_nc(E=1024, S=2048, SK=1024):
    key = (E, S, SK)
    if key in _NC_CACHE:
        return _NC_CACHE[key]
    import concourse.bacc as bacc

    nc = bacc.Bacc(None, target_bir_lowering=False)
    _emit(nc, E=E, S=S, SK=SK)
    nc.finalize()
    _NC_CACHE[key] = nc
    return nc


def _round_f32r(a):
    """Round fp32 to fp32r (tf32-like: 11 explicit mantissa bits, RNE)."""
    u = np.ascontiguousarray(a, np.float32).view(np.uint32)
    u = u + np.uint32(0x7FF) + ((u >> np.uint32(12)) & np.uint32(1))
    return (u & np.uint32(0xFFFFF000)).view(np.float32)


def make_in_maps(x, Wq, bq, Wk, bk, Wv, bv, E=1024, S=2048, SK=1024):
    """Host-side prep: per-core input dicts for run_bass_kernel_spmd."""
    import ml_dtypes

    bf16 = ml_dtypes.bfloat16
    ET = E // P
    scale = np.float32(1.0 / np.sqrt(np.float32(E)))
    x = np.asarray(x, np.float32)
    B = x.shape[0]
    n_half = S // SK

    def wtile(w):  # [f_tile, p(e), e_tile, c(f)] stationary blocks
        return np.ascontiguousarray(
            np.asarray(w, np.float32).reshape(ET, P, ET, P).transpose(0, 3, 2, 1)
        ).astype(bf16)

    wq8 = wtile(np.asarray(Wq, np.float32) * scale)
    wk8 = wtile(Wk)
    # wv8[e, p, f] = Wv[f, e*128+p]
    wv8 = np.ascontiguousarray(
        np.asarray(Wv, np.float32).T.reshape(ET, P, E)
    ).astype(bf16)
    bq8 = np.ascontiguousarray((np.asarray(bq, np.float32) * scale).reshape(ET, P).T)
    bk8 = np.ascontiguousarray(np.asarray(bk, np.float32).reshape(ET, P).T)
    ones8 = np.ones((P, ET), bf16)

    in_maps = []
    for c in range(B * n_half):
        b, h = divmod(c, n_half)
        xt_full = x[b].T  # [E, S]
        if h == 1:
            xt_full = np.concatenate([xt_full[:, SK:], xt_full[:, :SK]], axis=1)
        xt8 = np.ascontiguousarray(xt_full.reshape(ET, P, S)).astype(bf16)
        in_maps.append(
            {
                "xt8": xt8,
                "wq8": wq8,
                "wk8": wk8,
                "wv8": wv8,
                "bq8": bq8,
                "bk8": bk8,
                "ones8": ones8,
            }
        )
    return in_maps


def kernel(x, Wq, bq, Wk, bk, Wv, bv):
    from concourse.bass_utils import run_bass_kernel_spmd

    E, S, SK = 1024, 2048, 1024
    x = np.asarray(x, np.float32)
    B = x.shape[0]
    n_half = S // SK
    nc = build_nc(E=E, S=S, SK=SK)
    in_maps = make_in_maps(x, Wq, bq, Wk, bk, Wv, bv, E=E, S=S, SK=SK)
    n_cores = len(in_maps)
    res = run_bass_kernel_spmd(nc, in_maps, list(range(n_cores)))

    bvf = np.asarray(bv, np.float32)
    out = np.empty((B, S, E), np.float32)
    for b in range(B):
        osum = None
        rsum = None
        for h in range(n_half):
            r = res.results[b * n_half + h]
            o_h = np.asarray(r["ou"]).astype(np.float32)
            rs_h = np.asarray(r["rs"]).astype(np.float32).reshape(S)
            if h == 1:  # undo the query permutation
                o_h = np.concatenate([o_h[SK:], o_h[:SK]], axis=0)
                rs_h = np.concatenate([rs_h[SK:], rs_h[:SK]])
            osum = o_h if osum is None else osum + o_h
            rsum = rs_h if rsum is None else rsum + rs_h
        out[b] = osum / rsum[:, None] + bvf[None, :]
    return out


# revision 8
# speedup vs baseline: 1.3677x; 1.1251x over previous
"""Single-head attention (B=4, S=2048, E=1024, fp32) on 8 trn2 NeuronCores.

Sharding: (batch, key-half) -> 8 shards. Core c handles batch c//2 and the
key/value rows [h*1024, (h+1)*1024) with h = c%2. Each core computes the Q
projection for ALL 2048 queries of its batch, K/V projections for its own
1024 keys, exp(scores^T) against those keys, the unnormalized partial output
O_h = exp(S^T)^T @ V_h and the partial softmax denominators rs_h. The host
combines: out = (O_0 + O_1) / (rs_0 + rs_1) + bv  (the V bias commutes with
the softmax average, so it is added once on the host).

Dtype split: STATIONARY matmul operands are fp32r (standard 2-XBUS
LDWEIGHTS hides behind the moving stream; bf16 FWL grabs all 4 XBUSes and
serializes ~45ns/matmul), MOVING operands are bf16 (halves SBUF + DMA).
Exception: the V projection's stationary is the bf16 xt tile (cheaper than
keeping a second fp32r copy of x).

  xt [128, 8e, 2048] bf16  x[b]^T, key-half columns first (host permute).
  w  [128, 8e, 128] f32r   Wq^T*scale / Wk^T stationary tiles (streamed).
  qt [128, 8f, 2048] bf16  Q^T - moving operand of scores.
  kt [128, 8f, 1024] f32r  K^T - stationary of scores.
  wv [128, 8e, 1024] bf16  Wv^T - moving operand of the V projection.
  v  [128, 8k, 1024] bf16  V - moving operand of O.
  pt [128, 8k, 2048] f32r  exp(S^T) - stationary of O, moving of rowsum.

Rowsums come from ones^T @ exp tiles on the PE. A burst of tiny warmup
matmuls runs during the initial input DMA so the PE's activity-based clock
ramp (1.2 -> 2.4 GHz) completes before the first real matmul.
"""

import numpy as np

P = 128


def _emit(nc, E=1024, S=2048, SK=1024):
    import concourse.mybir as mybir
    import concourse.tile as tile

    f32 = mybir.dt.float32
    f32r = mybir.dt.float32r
    bf16 = mybir.dt.bfloat16
    fp8 = mybir.dt.float8e4
    ACT = mybir.ActivationFunctionType

    ET = E // P     # e/f tiles (8)
    QT = S // P     # q tiles (16)
    KT = SK // P    # k tiles (8)
    NQC = S // 512  # q chunks (4)
    NKC = SK // 512  # k chunks (2)
    NFC = E // 512  # f chunks (2)

    xt8 = nc.dram_tensor("xt8", [ET, P, S], bf16, kind="ExternalInput")
    wq8 = nc.dram_tensor("wq8", [ET, P, ET, P], bf16, kind="ExternalInput")
    wk8 = nc.dram_tensor("wk8", [ET, P, ET, P], bf16, kind="ExternalInput")
    wv8 = nc.dram_tensor("wv8", [ET, P, E], bf16, kind="ExternalInput")
    bq8 = nc.dram_tensor("bq8", [P, ET], f32, kind="ExternalInput")
    bk8 = nc.dram_tensor("bk8", [P, ET], f32, kind="ExternalInput")
    ones8 = nc.dram_tensor("ones8", [P, ET], bf16, kind="ExternalInput")
    ou = nc.dram_tensor("ou", [S, E], bf16, kind="ExternalOutput")
    rs = nc.dram_tensor("rs", [1, S], f32, kind="ExternalOutput")

    with tile.TileContext(nc) as tc:
        with (
            tc.tile_pool(name="psum", bufs=6, space="PSUM") as psum,
            tc.tile_pool(name="small", bufs=1) as small,
            tc.tile_pool(name="persist", bufs=1) as pers,
            tc.tile_pool(name="obuf", bufs=3) as obp,
        ):
            ones_t = small.tile([P, ET], bf16, tag="ones")
            nc.sync.dma_start(ones_t[:], ones8[:])
            bq_t = small.tile([P, ET], f32, tag="bq")
            nc.sync.dma_start(bq_t[:], bq8[:])
            bk_t = small.tile([P, ET], f32, tag="bk")
            nc.sync.dma_start(bk_t[:], bk8[:])
            rs_sb = small.tile([1, S], f32, tag="rssb")

            qt_t = pers.tile([P, ET, S], fp8, tag="qt")
            kt_t = pers.tile([P, ET, SK], fp8, tag="kt")
            v_t = pers.tile([P, KT, E], bf16, tag="v")

            rs_ps = [
                psum.tile([1, 512], f32, tag=f"rs{qc}", name=f"rs{qc}", bufs=1)
                for qc in range(2)
            ]

            with (
                tc.tile_pool(name="ph1", bufs=1) as ph1,
                tc.tile_pool(name="wstream", bufs=3) as wsp,
            ):
                xt_t = ph1.tile([P, ET, S], bf16, tag="xt")
                wv_t = ph1.tile([P, ET, E], bf16, tag="wv")

                # ---- input DMA: sync ring = weights; xt alternates over the
                # gpsimd + vector rings, key-half columns first ----
                wk_rows = []
                w_t = wsp.tile([P, ET, P], bf16, tag="w", name="wk_f0")
                nc.sync.dma_start(w_t[:], wk8[0])
                wk_rows.append(w_t)
                xt_rings = [nc.gpsimd, nc.scalar]
                for kc in range(NKC):  # key half, chunk-sized for early starts
                    for e in range(ET):
                        xt_rings[e % 2].dma_start(
                            xt_t[:, e, kc * 512 : (kc + 1) * 512],
                            xt8[e, :, kc * 512 : (kc + 1) * 512],
                        )
                for f in range(1, ET):
                    w_t = wsp.tile([P, ET, P], bf16, tag="w", name=f"wk_f{f}")
                    nc.sync.dma_start(w_t[:], wk8[f])
                    wk_rows.append(w_t)
                for e in range(ET):  # q-only columns, one wide DMA each
                    xt_rings[e % 2].dma_start(xt_t[:, e, SK:S], xt8[e, :, SK:S])
                wq_rows = []
                for f in range(ET):
                    w_t = wsp.tile([P, ET, P], bf16, tag="w", name=f"wq_f{f}")
                    nc.sync.dma_start(w_t[:], wq8[f])
                    wq_rows.append(w_t)
                for e in range(ET):
                    nc.scalar.dma_start(wv_t[:, e], wv8[e])

                # ---- PE warmup (HAM clock ramp) while input DMAs stream ----
                for i in range(48):
                    nc.tensor.matmul(
                        rs_ps[0][:, 0:8], ones_t[:, 0:1], ones_t[:],
                        start=True, stop=True,
                    )

                # ---- K projection (key half = first SK columns of xt) ----
                for f in range(ET):
                    ps2 = [
                        psum.tile([P, 512], f32, tag="mm", name=f"k{f}_{kc}")
                        for kc in range(NKC)
                    ]
                    for e in range(ET):
                        for kc in range(NKC):
                            nc.tensor.matmul(
                                ps2[kc][:],
                                wk_rows[f][:, e],
                                xt_t[:, e, kc * 512 : (kc + 1) * 512],
                                start=(e == 0),
                                stop=(e == ET - 1),
                            )
                    for kc in range(NKC):
                        nc.scalar.add(
                            kt_t[:, f, kc * 512 : (kc + 1) * 512],
                            ps2[kc][:],
                            bk_t[:, f : f + 1],
                        )

                # ---- Q projection over all queries ----
                for f in range(ET):
                    ps4 = [
                        psum.tile([P, 512], f32, tag="mm", name=f"q{f}_{qc}")
                        for qc in range(NQC)
                    ]
                    for e in range(ET):
                        for qc in range(NQC):
                            nc.tensor.matmul(
                                ps4[qc][:],
                                wq_rows[f][:, e],
                                xt_t[:, e, qc * 512 : (qc + 1) * 512],
                                start=(e == 0),
                                stop=(e == ET - 1),
                            )
                    for qc in range(NQC):
                        nc.scalar.add(
                            qt_t[:, f, qc * 512 : (qc + 1) * 512],
                            ps4[qc][:],
                            bq_t[:, f : f + 1],
                        )

                # ---- V projection: v[k, f] = sum_e xt[e, k] * wv[e, f] ----
                for kt in range(KT):
                    ps2 = [
                        psum.tile([P, 512], f32, tag="mm", name=f"v{kt}_{fc}")
                        for fc in range(NFC)
                    ]
                    for e in range(ET):
                        for fc in range(NFC):
                            nc.tensor.matmul(
                                ps2[fc][:],
                                xt_t[:, e, kt * P : (kt + 1) * P],
                                wv_t[:, e, fc * 512 : (fc + 1) * 512],
                                start=(e == 0),
                                stop=(e == ET - 1),
                            )
                    for fc in range(NFC):
                        nc.vector.tensor_copy(
                            v_t[:, kt, fc * 512 : (fc + 1) * 512], ps2[fc][:]
                        )

            with tc.tile_pool(name="ptp", bufs=1) as ptp:
                pt_t = ptp.tile([P, KT, S], bf16, tag="pt")

                # ---- scores^T (fp8 DoubleRow) + exp; rowsums in a second
                # pass so the PE never waits on the scalar exp ----
                DR = mybir.MatmulPerfMode.DoubleRow
                scale = float(1.0 / np.sqrt(np.float32(E)))
                for qh in range(2):
                    for kt in range(KT):
                        ps2 = [
                            psum.tile([P, 512], f32, tag="mm", name=f"s{kt}_{qc}")
                            for qc in range(2)
                        ]
                        for fp in range(ET // 2):
                            for qc in range(2):
                                col = qh * 1024 + qc * 512
                                nc.tensor.matmul(
                                    ps2[qc][:],
                                    kt_t[:, 2 * fp : 2 * fp + 2, kt * P : (kt + 1) * P],
                                    qt_t[:, 2 * fp : 2 * fp + 2, col : col + 512],
                                    start=(fp == 0),
                                    stop=(fp == ET // 2 - 1),
                                    perf_mode=DR,
                                )
                        for qc in range(2):
                            col = qh * 1024 + qc * 512
                            nc.scalar.activation(
                                pt_t[:, kt, col : col + 512], ps2[qc][:], ACT.Exp,
                                scale=scale,
                            )
                    for kt in range(KT):
                        for qc in range(2):
                            col = qh * 1024 + qc * 512
                            nc.tensor.matmul(
                                rs_ps[qc][:],
                                ones_t[:, 0:1],
                                pt_t[:, kt, col : col + 512],
                                start=(kt == 0),
                                stop=(kt == KT - 1),
                            )
                    for qc in range(2):
                        col = qh * 1024 + qc * 512
                        nc.vector.tensor_copy(
                            rs_sb[:, col : col + 512], rs_ps[qc][:]
                        )

                # ---- O = pt^T @ v, unnormalized; store bf16 ----
                for qt in range(QT):
                    po = [
                        psum.tile([P, 512], f32, tag="mm", name=f"o{qt}_{fc}")
                        for fc in range(NFC)
                    ]
                    for kt in range(KT):
                        for fc in range(NFC):
                            nc.tensor.matmul(
                                po[fc][:],
                                pt_t[:, kt, qt * P : (qt + 1) * P],
                                v_t[:, kt, fc * 512 : (fc + 1) * 512],
                                start=(kt == 0),
                                stop=(kt == KT - 1),
                            )
                    o_sb = obp.tile([P, E], bf16, tag="ob")
                    for fc in range(NFC):
                        nc.vector.tensor_copy(
                            o_sb[:, fc * 512 : (fc + 1) * 512], po[fc][:]
                        )
                    nc.gpsimd.dma_start(ou[qt * P : (qt + 1) * P, :], o_sb[:])
                nc.sync.dma_start(rs[:], rs_sb[:])


_NC_CACHE = {}


def build_nc(E=1024, S=2048, SK=1024):
    key = (E, S, SK)
    if key in _NC_CACHE:
        return _NC_CACHE[key]
    import concourse.bacc as bacc

    nc = bacc.Bacc(None, target_bir_lowering=False)
    _emit(nc, E=E, S=S, SK=SK)
    nc.finalize()
    _NC_CACHE[key] = nc
    return nc


def _round_f32r(a):
    """Round fp32 to fp32r (tf32-like: 11 explicit mantissa bits, RNE)."""
    u = np.ascontiguousarray(a, np.float32).view(np.uint32)
    u = u + np.uint32(0x7FF) + ((u >> np.uint32(12)) & np.uint32(1))
    return (u & np.uint32(0xFFFFF000)).view(np.float32)


def make_in_maps(x, Wq, bq, Wk, bk, Wv, bv, E=1024, S=2048, SK=1024):
    """Host-side prep: per-core input dicts for run_bass_kernel_spmd."""
    import ml_dtypes

    bf16 = ml_dtypes.bfloat16
    ET = E // P
    scale = np.float32(1.0 / np.sqrt(np.float32(E)))
    x = np.asarray(x, np.float32)
    B = x.shape[0]
    n_half = S // SK

    def wtile(w):  # [f_tile, p(e), e_tile, c(f)] stationary blocks
        return np.ascontiguousarray(
            np.asarray(w, np.float32).reshape(ET, P, ET, P).transpose(0, 3, 2, 1)
        ).astype(bf16)

    wq8 = wtile(Wq)
    wk8 = wtile(Wk)
    # wv8[e, p, f] = Wv[f, e*128+p]
    wv8 = np.ascontiguousarray(
        np.asarray(Wv, np.float32).T.reshape(ET, P, E)
    ).astype(bf16)
    bq8 = np.ascontiguousarray(np.asarray(bq, np.float32).reshape(ET, P).T)
    bk8 = np.ascontiguousarray(np.asarray(bk, np.float32).reshape(ET, P).T)
    ones8 = np.ones((P, ET), bf16)

    in_maps = []
    for c in range(B * n_half):
        b, h = divmod(c, n_half)
        xt_full = x[b].T  # [E, S]
        if h == 1:
            xt_full = np.concatenate([xt_full[:, SK:], xt_full[:, :SK]], axis=1)
        xt8 = np.ascontiguousarray(xt_full.reshape(ET, P, S)).astype(bf16)
        in_maps.append(
            {
                "xt8": xt8,
                "wq8": wq8,
                "wk8": wk8,
                "wv8": wv8,
                "bq8": bq8,
                "bk8": bk8,
                "ones8": ones8,
            }
        )
    return in_maps


def kernel(x, Wq, bq, Wk, bk, Wv, bv):
    from concourse.bass_utils import run_bass_kernel_spmd

    E, S, SK = 1024, 2048, 1024
    x = np.asarray(x, np.float32)
    B = x.shape[0]
    n_half = S // SK
    nc = build_nc(E=E, S=S, SK=SK)
    in_maps = make_in_maps(x, Wq, bq, Wk, bk, Wv, bv, E=E, S=S, SK=SK)
    n_cores = len(in_maps)
    res = run_bass_kernel_spmd(nc, in_maps, list(range(n_cores)))

    bvf = np.asarray(bv, np.float32)
    out = np.empty((B, S, E), np.float32)
    for b in range(B):
        osum = None
        rsum = None
        for h in range(n_half):
            r = res.results[b * n_half + h]
            o_h = np.asarray(r["ou"]).astype(np.float32)
            rs_h = np.asarray(r["rs"]).astype(np.float32).reshape(S)
            if h == 1:  # undo the query permutation
                o_h = np.concatenate([o_h[SK:], o_h[:SK]], axis=0)
                rs_h = np.concatenate([rs_h[SK:], rs_h[:SK]])
            osum = o_h if osum is None else osum + o_h
            rsum = rs_h if rsum is None else rsum + rs_h
        out[b] = osum / rsum[:, None] + bvf[None, :]
    return out


# revision 9
# speedup vs baseline: 1.3873x; 1.0143x over previous
"""Single-head attention (B=4, S=2048, E=1024, fp32) on 8 trn2 NeuronCores.

Sharding: (batch, key-half) -> 8 shards. Core c handles batch c//2 and the
key/value rows [h*1024, (h+1)*1024) with h = c%2. Each core computes the Q
projection for ALL 2048 queries of its batch, K/V projections for its own
1024 keys, exp(scores^T) against those keys, the unnormalized partial output
O_h = exp(S^T)^T @ V_h and the partial softmax denominators rs_h. The host
combines: out = (O_0 + O_1) / (rs_0 + rs_1) + bv  (the V bias commutes with
the softmax average, so it is added once on the host).

Dtype split: STATIONARY matmul operands are fp32r (standard 2-XBUS
LDWEIGHTS hides behind the moving stream; bf16 FWL grabs all 4 XBUSes and
serializes ~45ns/matmul), MOVING operands are bf16 (halves SBUF + DMA).
Exception: the V projection's stationary is the bf16 xt tile (cheaper than
keeping a second fp32r copy of x).

  xt [128, 8e, 2048] bf16  x[b]^T, key-half columns first (host permute).
  w  [128, 8e, 128] f32r   Wq^T*scale / Wk^T stationary tiles (streamed).
  qt [128, 8f, 2048] bf16  Q^T - moving operand of scores.
  kt [128, 8f, 1024] f32r  K^T - stationary of scores.
  wv [128, 8e, 1024] bf16  Wv^T - moving operand of the V projection.
  v  [128, 8k, 1024] bf16  V - moving operand of O.
  pt [128, 8k, 2048] f32r  exp(S^T) - stationary of O, moving of rowsum.

Rowsums come from ones^T @ exp tiles on the PE. A burst of tiny warmup
matmuls runs during the initial input DMA so the PE's activity-based clock
ramp (1.2 -> 2.4 GHz) completes before the first real matmul.
"""

import numpy as np

P = 128


def _emit(nc, E=1024, S=2048, SK=1024):
    import concourse.mybir as mybir
    import concourse.tile as tile

    f32 = mybir.dt.float32
    f32r = mybir.dt.float32r
    bf16 = mybir.dt.bfloat16
    fp8 = mybir.dt.float8e4
    ACT = mybir.ActivationFunctionType

    ET = E // P     # e/f tiles (8)
    QT = S // P     # q tiles (16)
    KT = SK // P    # k tiles (8)
    NQC = S // 512  # q chunks (4)
    NKC = SK // 512  # k chunks (2)
    NFC = E // 512  # f chunks (2)

    xt8 = nc.dram_tensor("xt8", [ET, P, S], bf16, kind="ExternalInput")
    wq8 = nc.dram_tensor("wq8", [ET, P, ET, P], bf16, kind="ExternalInput")
    wk8 = nc.dram_tensor("wk8", [ET, P, ET, P], bf16, kind="ExternalInput")
    wv8 = nc.dram_tensor("wv8", [ET, P, E], bf16, kind="ExternalInput")
    bq8 = nc.dram_tensor("bq8", [P, ET], f32, kind="ExternalInput")
    bk8 = nc.dram_tensor("bk8", [P, ET], f32, kind="ExternalInput")
    ones8 = nc.dram_tensor("ones8", [P, 512], bf16, kind="ExternalInput")
    ou = nc.dram_tensor("ou", [S, E], bf16, kind="ExternalOutput")
    rs = nc.dram_tensor("rs", [1, S], f32, kind="ExternalOutput")

    with tile.TileContext(nc) as tc:
        with (
            tc.tile_pool(name="psum", bufs=6, space="PSUM") as psum,
            tc.tile_pool(name="small", bufs=1) as small,
            tc.tile_pool(name="persist", bufs=1) as pers,
            tc.tile_pool(name="obuf", bufs=3) as obp,
        ):
            ones_t = small.tile([P, 512], bf16, tag="ones")
            nc.gpsimd.dma_start(ones_t[:], ones8[:])
            bq_t = small.tile([P, ET], f32, tag="bq")
            nc.sync.dma_start(bq_t[:], bq8[:])
            bk_t = small.tile([P, ET], f32, tag="bk")
            nc.sync.dma_start(bk_t[:], bk8[:])
            rs_sb = small.tile([1, S], f32, tag="rssb")

            qt_t = pers.tile([P, ET, S], fp8, tag="qt")
            kt_t = pers.tile([P, ET, SK], fp8, tag="kt")
            v_t = pers.tile([P, KT, E], bf16, tag="v")

            rs_ps = [
                psum.tile([1, 512], f32, tag=f"rs{qc}", name=f"rs{qc}", bufs=1)
                for qc in range(2)
            ]

            with (
                tc.tile_pool(name="ph1", bufs=1) as ph1,
                tc.tile_pool(name="wstream", bufs=3) as wsp,
            ):
                xt_t = ph1.tile([P, ET, S], bf16, tag="xt")
                wv_t = ph1.tile([P, ET, E], bf16, tag="wv")

                # ---- input DMA: sync ring = weights; xt alternates over the
                # gpsimd + vector rings, key-half columns first ----
                wk_rows = []
                w_t = wsp.tile([P, ET, P], bf16, tag="w", name="wk_f0")
                nc.sync.dma_start(w_t[:], wk8[0])
                wk_rows.append(w_t)
                xt_rings = [nc.gpsimd, nc.scalar]
                for kc in range(NKC):  # key half, chunk-sized for early starts
                    for e in range(ET):
                        xt_rings[e % 2].dma_start(
                            xt_t[:, e, kc * 512 : (kc + 1) * 512],
                            xt8[e, :, kc * 512 : (kc + 1) * 512],
                        )
                for f in range(1, ET):
                    w_t = wsp.tile([P, ET, P], bf16, tag="w", name=f"wk_f{f}")
                    nc.sync.dma_start(w_t[:], wk8[f])
                    wk_rows.append(w_t)
                for e in range(ET):  # q-only columns, one wide DMA each
                    xt_rings[e % 2].dma_start(xt_t[:, e, SK:S], xt8[e, :, SK:S])
                wq_rows = []
                for f in range(ET):
                    w_t = wsp.tile([P, ET, P], bf16, tag="w", name=f"wq_f{f}")
                    nc.sync.dma_start(w_t[:], wq8[f])
                    wq_rows.append(w_t)
                for e in range(ET):
                    nc.scalar.dma_start(wv_t[:, e], wv8[e])

                # ---- PE warmup (HAM clock ramp) while input DMAs stream:
                # full-width matmuls span ~6us, bridging to first data ----
                for i in range(14):
                    nc.tensor.matmul(
                        rs_ps[0][:], ones_t[:, 0:1], ones_t[:],
                        start=True, stop=True,
                    )

                # ---- K projection (key half = first SK columns of xt) ----
                for f in range(ET):
                    ps2 = [
                        psum.tile([P, 512], f32, tag="mm", name=f"k{f}_{kc}")
                        for kc in range(NKC)
                    ]
                    for e in range(ET):
                        for kc in range(NKC):
                            nc.tensor.matmul(
                                ps2[kc][:],
                                wk_rows[f][:, e],
                                xt_t[:, e, kc * 512 : (kc + 1) * 512],
                                start=(e == 0),
                                stop=(e == ET - 1),
                            )
                    for kc in range(NKC):
                        nc.scalar.add(
                            kt_t[:, f, kc * 512 : (kc + 1) * 512],
                            ps2[kc][:],
                            bk_t[:, f : f + 1],
                        )

                # ---- Q projection over all queries ----
                for f in range(ET):
                    ps4 = [
                        psum.tile([P, 512], f32, tag="mm", name=f"q{f}_{qc}")
                        for qc in range(NQC)
                    ]
                    for e in range(ET):
                        for qc in range(NQC):
                            nc.tensor.matmul(
                                ps4[qc][:],
                                wq_rows[f][:, e],
                                xt_t[:, e, qc * 512 : (qc + 1) * 512],
                                start=(e == 0),
                                stop=(e == ET - 1),
                            )
                    for qc in range(NQC):
                        nc.scalar.add(
                            qt_t[:, f, qc * 512 : (qc + 1) * 512],
                            ps4[qc][:],
                            bq_t[:, f : f + 1],
                        )

                # ---- V projection: v[k, f] = sum_e xt[e, k] * wv[e, f] ----
                for kt in range(KT):
                    ps2 = [
                        psum.tile([P, 512], f32, tag="mm", name=f"v{kt}_{fc}")
                        for fc in range(NFC)
                    ]
                    for e in range(ET):
                        for fc in range(NFC):
                            nc.tensor.matmul(
                                ps2[fc][:],
                                xt_t[:, e, kt * P : (kt + 1) * P],
                                wv_t[:, e, fc * 512 : (fc + 1) * 512],
                                start=(e == 0),
                                stop=(e == ET - 1),
                            )
                    for fc in range(NFC):
                        nc.vector.tensor_copy(
                            v_t[:, kt, fc * 512 : (fc + 1) * 512], ps2[fc][:]
                        )

            with tc.tile_pool(name="ptp", bufs=1) as ptp:
                pt_t = ptp.tile([P, KT, S], bf16, tag="pt")

                # ---- scores^T (fp8 DoubleRow) + exp; rowsums in a second
                # pass so the PE never waits on the scalar exp ----
                DR = mybir.MatmulPerfMode.DoubleRow
                scale = float(1.0 / np.sqrt(np.float32(E)))
                for qh in range(2):
                    for kt in range(KT):
                        ps2 = [
                            psum.tile([P, 512], f32, tag="mm", name=f"s{kt}_{qc}")
                            for qc in range(2)
                        ]
                        for fp in range(ET // 2):
                            for qc in range(2):
                                col = qh * 1024 + qc * 512
                                nc.tensor.matmul(
                                    ps2[qc][:],
                                    kt_t[:, 2 * fp : 2 * fp + 2, kt * P : (kt + 1) * P],
                                    qt_t[:, 2 * fp : 2 * fp + 2, col : col + 512],
                                    start=(fp == 0),
                                    stop=(fp == ET // 2 - 1),
                                    perf_mode=DR,
                                )
                        for qc in range(2):
                            col = qh * 1024 + qc * 512
                            nc.scalar.activation(
                                pt_t[:, kt, col : col + 512], ps2[qc][:], ACT.Exp,
                                scale=scale,
                            )
                    for kt in range(KT):
                        for qc in range(2):
                            col = qh * 1024 + qc * 512
                            nc.tensor.matmul(
                                rs_ps[qc][:],
                                ones_t[:, 0:1],
                                pt_t[:, kt, col : col + 512],
                                start=(kt == 0),
                                stop=(kt == KT - 1),
                            )
                    for qc in range(2):
                        col = qh * 1024 + qc * 512
                        nc.vector.tensor_copy(
                            rs_sb[:, col : col + 512], rs_ps[qc][:]
                        )

                # ---- O = pt^T @ v, unnormalized; store bf16 ----
                for qt in range(QT):
                    po = [
                        psum.tile([P, 512], f32, tag="mm", name=f"o{qt}_{fc}")
                        for fc in range(NFC)
                    ]
                    for kt in range(KT):
                        for fc in range(NFC):
                            nc.tensor.matmul(
                                po[fc][:],
                                pt_t[:, kt, qt * P : (qt + 1) * P],
                                v_t[:, kt, fc * 512 : (fc + 1) * 512],
                                start=(kt == 0),
                                stop=(kt == KT - 1),
                            )
                    o_sb = obp.tile([P, E], bf16, tag="ob")
                    for fc in range(NFC):
                        nc.vector.tensor_copy(
                            o_sb[:, fc * 512 : (fc + 1) * 512], po[fc][:]
                        )
                    nc.gpsimd.dma_start(ou[qt * P : (qt + 1) * P, :], o_sb[:])
                nc.sync.dma_start(rs[:], rs_sb[:])


_NC_CACHE = {}


def build_nc(E=1024, S=2048, SK=1024):
    key = (E, S, SK)
    if key in _NC_CACHE:
        return _NC_CACHE[key]
    import concourse.bacc as bacc

    nc = bacc.Bacc(None, target_bir_lowering=False)
    _emit(nc, E=E, S=S, SK=SK)
    nc.finalize()
    _NC_CACHE[key] = nc
    return nc


def _round_f32r(a):
    """Round fp32 to fp32r (tf32-like: 11 explicit mantissa bits, RNE)."""
    u = np.ascontiguousarray(a, np.float32).view(np.uint32)
    u = u + np.uint32(0x7FF) + ((u >> np.uint32(12)) & np.uint32(1))
    return (u & np.uint32(0xFFFFF000)).view(np.float32)


def make_in_maps(x, Wq, bq, Wk, bk, Wv, bv, E=1024, S=2048, SK=1024):
    """Host-side prep: per-core input dicts for run_bass_kernel_spmd."""
    import ml_dtypes

    bf16 = ml_dtypes.bfloat16
    ET = E // P
    scale = np.float32(1.0 / np.sqrt(np.float32(E)))
    x = np.asarray(x, np.float32)
    B = x.shape[0]
    n_half = S // SK

    def wtile(w):  # [f_tile, p(e), e_tile, c(f)] stationary blocks
        return np.ascontiguousarray(
            np.asarray(w, np.float32).reshape(ET, P, ET, P).transpose(0, 3, 2, 1)
        ).astype(bf16)

    wq8 = wtile(Wq)
    wk8 = wtile(Wk)
    # wv8[e, p, f] = Wv[f, e*128+p]
    wv8 = np.ascontiguousarray(
        np.asarray(Wv, np.float32).T.reshape(ET, P, E)
    ).astype(bf16)
    bq8 = np.ascontiguousarray(np.asarray(bq, np.float32).reshape(ET, P).T)
    bk8 = np.ascontiguousarray(np.asarray(bk, np.float32).reshape(ET, P).T)
    ones8 = np.ones((P, 512), bf16)

    in_maps = []
    for c in range(B * n_half):
        b, h = divmod(c, n_half)
        xt_full = x[b].T  # [E, S]
        if h == 1:
            xt_full = np.concatenate([xt_full[:, SK:], xt_full[:, :SK]], axis=1)
        xt8 = np.ascontiguousarray(xt_full.reshape(ET, P, S)).astype(bf16)
        in_maps.append(
            {
                "xt8": xt8,
                "wq8": wq8,
                "wk8": wk8,
                "wv8": wv8,
                "bq8": bq8,
                "bk8": bk8,
                "ones8": ones8,
            }
        )
    return in_maps


def kernel(x, Wq, bq, Wk, bk, Wv, bv):
    from concourse.bass_utils import run_bass_kernel_spmd

    E, S, SK = 1024, 2048, 1024
    x = np.asarray(x, np.float32)
    B = x.shape[0]
    n_half = S // SK
    nc = build_nc(E=E, S=S, SK=SK)
    in_maps = make_in_maps(x, Wq, bq, Wk, bk, Wv, bv, E=E, S=S, SK=SK)
    n_cores = len(in_maps)
    res = run_bass_kernel_spmd(nc, in_maps, list(range(n_cores)))

    bvf = np.asarray(bv, np.float32)
    out = np.empty((B, S, E), np.float32)
    for b in range(B):
        osum = None
        rsum = None
        for h in range(n_half):
            r = res.results[b * n_half + h]
            o_h = np.asarray(r["ou"]).astype(np.float32)
            rs_h = np.asarray(r["rs"]).astype(np.float32).reshape(S)
            if h == 1:  # undo the query permutation
                o_h = np.concatenate([o_h[SK:], o_h[:SK]], axis=0)
                rs_h = np.concatenate([rs_h[SK:], rs_h[:SK]])
            osum = o_h if osum is None else osum + o_h
            rsum = rs_h if rsum is None else rsum + rs_h
        out[b] = osum / rsum[:, None] + bvf[None, :]
    return out


# revision 10
# speedup vs baseline: 1.4795x; 1.0664x over previous
"""Single-head attention (B=4, S=2048, E=1024, fp32) on 8 trn2 NeuronCores.

Sharding: (batch, key-half) -> 8 shards. Core c handles batch c//2 and the
key/value rows [h*1024, (h+1)*1024) with h = c%2. Each core computes the Q
projection for ALL 2048 queries of its batch, K/V projections for its own
1024 keys, exp(scores^T) against those keys, the unnormalized partial output
O_h = exp(S^T)^T @ V_h and the partial softmax denominators rs_h. The host
combines: out = (O_0 + O_1) / (rs_0 + rs_1) + bv  (the V bias commutes with
the softmax average, so it is added once on the host).

Dtype split: STATIONARY matmul operands are fp32r (standard 2-XBUS
LDWEIGHTS hides behind the moving stream; bf16 FWL grabs all 4 XBUSes and
serializes ~45ns/matmul), MOVING operands are bf16 (halves SBUF + DMA).
Exception: the V projection's stationary is the bf16 xt tile (cheaper than
keeping a second fp32r copy of x).

  xt [128, 8e, 2048] bf16  x[b]^T, key-half columns first (host permute).
  w  [128, 8e, 128] f32r   Wq^T*scale / Wk^T stationary tiles (streamed).
  qt [128, 8f, 2048] bf16  Q^T - moving operand of scores.
  kt [128, 8f, 1024] f32r  K^T - stationary of scores.
  wv [128, 8e, 1024] bf16  Wv^T - moving operand of the V projection.
  v  [128, 8k, 1024] bf16  V - moving operand of O.
  pt [128, 8k, 2048] f32r  exp(S^T) - stationary of O, moving of rowsum.

Rowsums come from ones^T @ exp tiles on the PE. A burst of tiny warmup
matmuls runs during the initial input DMA so the PE's activity-based clock
ramp (1.2 -> 2.4 GHz) completes before the first real matmul.
"""

import numpy as np

P = 128


def _emit(nc, E=1024, S=2048, SK=1024):
    import concourse.mybir as mybir
    import concourse.tile as tile

    f32 = mybir.dt.float32
    f32r = mybir.dt.float32r
    bf16 = mybir.dt.bfloat16
    fp8 = mybir.dt.float8e4
    ACT = mybir.ActivationFunctionType

    ET = E // P     # e/f tiles (8)
    QT = S // P     # q tiles (16)
    KT = SK // P    # k tiles (8)
    NQC = S // 512  # q chunks (4)
    NKC = SK // 512  # k chunks (2)
    NFC = E // 512  # f chunks (2)

    xt8 = nc.dram_tensor("xt8", [ET, P, S], bf16, kind="ExternalInput")
    wq8 = nc.dram_tensor("wq8", [ET, P, ET, P], bf16, kind="ExternalInput")
    wk8 = nc.dram_tensor("wk8", [ET, P, ET, P], bf16, kind="ExternalInput")
    wv8 = nc.dram_tensor("wv8", [ET, P, E], bf16, kind="ExternalInput")
    bq8 = nc.dram_tensor("bq8", [P, ET], f32, kind="ExternalInput")
    bk8 = nc.dram_tensor("bk8", [P, ET], f32, kind="ExternalInput")
    ones8 = nc.dram_tensor("ones8", [P, 512], bf16, kind="ExternalInput")
    ou = nc.dram_tensor("ou", [S, E], bf16, kind="ExternalOutput")
    rs = nc.dram_tensor("rs", [1, S], f32, kind="ExternalOutput")

    groups = [[2 * i, 2 * i + 1] for i in range(4)]

    with tile.TileContext(nc) as tc:
        with (
            tc.tile_pool(name="dramp", bufs=1, space="DRAM") as dramp,
            tc.tile_pool(name="psum", bufs=6, space="PSUM") as psum,
            tc.tile_pool(name="small", bufs=1) as small,
            tc.tile_pool(name="persist", bufs=1) as pers,
            tc.tile_pool(name="obuf", bufs=3) as obp,
        ):
            ones_t = small.tile([P, 512], bf16, tag="ones")
            nc.gpsimd.dma_start(ones_t[:], ones8[:])
            bq_t = small.tile([P, ET], f32, tag="bq")
            nc.sync.dma_start(bq_t[:], bq8[:])
            bk_t = small.tile([P, ET], f32, tag="bk")
            nc.sync.dma_start(bk_t[:], bk8[:])
            rs_sb = small.tile([1, S], f32, tag="rssb")

            qt_g = pers.tile([P, ET, S], fp8, tag="qtg")
            kt_t = pers.tile([P, ET, SK], fp8, tag="kt")
            v_t = pers.tile([P, KT, E], bf16, tag="v")

            rs_ps = [
                psum.tile([1, 512], f32, tag=f"rs{qc}", name=f"rs{qc}", bufs=1)
                for qc in range(2)
            ]

            with (
                tc.tile_pool(name="ph1", bufs=1) as ph1,
                tc.tile_pool(name="wstream", bufs=3) as wsp,
            ):
                xt_t = ph1.tile([P, ET, S], bf16, tag="xt")
                wv_t = ph1.tile([P, ET, E], bf16, tag="wv")
                qt_t = ph1.tile([P, ET, SK], fp8, tag="qt")
                qh_d = dramp.tile([P, ET, SK], fp8, tag="qhd")
                qg_d = dramp.tile([2, P, ET, SK], fp8, tag="qgd")

                # ---- input DMA: sync ring = weights; xt alternates over the
                # gpsimd + vector rings, key-half columns first ----
                wk_rows = []
                w_t = wsp.tile([P, ET, P], bf16, tag="w", name="wk_f0")
                nc.sync.dma_start(w_t[:], wk8[0])
                wk_rows.append(w_t)
                xt_rings = [nc.gpsimd, nc.scalar]
                for kc in range(NKC):  # key half, chunk-sized for early starts
                    for e in range(ET):
                        xt_rings[e % 2].dma_start(
                            xt_t[:, e, kc * 512 : (kc + 1) * 512],
                            xt8[e, :, kc * 512 : (kc + 1) * 512],
                        )
                for f in range(1, ET):
                    w_t = wsp.tile([P, ET, P], bf16, tag="w", name=f"wk_f{f}")
                    nc.sync.dma_start(w_t[:], wk8[f])
                    wk_rows.append(w_t)
                for e in range(ET):  # q-only columns, one wide DMA each
                    xt_rings[e % 2].dma_start(xt_t[:, e, SK:S], xt8[e, :, SK:S])
                wq_rows = []
                for f in range(ET):
                    w_t = wsp.tile([P, ET, P], bf16, tag="w", name=f"wq_f{f}")
                    nc.sync.dma_start(w_t[:], wq8[f])
                    wq_rows.append(w_t)
                for e in range(ET):
                    nc.scalar.dma_start(wv_t[:, e], wv8[e])

                # ---- PE warmup (HAM clock ramp) while input DMAs stream:
                # full-width matmuls span ~6us, bridging to first data ----
                for i in range(14):
                    nc.tensor.matmul(
                        rs_ps[0][:], ones_t[:, 0:1], ones_t[:],
                        start=True, stop=True,
                    )

                # ---- K projection (key half = first SK columns of xt) ----
                for f in range(ET):
                    ps2 = [
                        psum.tile([P, 512], f32, tag="mm", name=f"k{f}_{kc}")
                        for kc in range(NKC)
                    ]
                    for e in range(ET):
                        for kc in range(NKC):
                            nc.tensor.matmul(
                                ps2[kc][:],
                                wk_rows[f][:, e],
                                xt_t[:, e, kc * 512 : (kc + 1) * 512],
                                start=(e == 0),
                                stop=(e == ET - 1),
                            )
                    for kc in range(NKC):
                        nc.scalar.add(
                            kt_t[:, f, kc * 512 : (kc + 1) * 512],
                            ps2[kc][:],
                            bk_t[:, f : f + 1],
                        )

                # ---- Q projection, own query half only (the first SK
                # permuted columns = global queries [h*SK,(h+1)*SK)) ----
                for f in range(ET):
                    ps4 = [
                        psum.tile([P, 512], f32, tag="mm", name=f"q{f}_{qc}")
                        for qc in range(NKC)
                    ]
                    for e in range(ET):
                        for qc in range(NKC):
                            nc.tensor.matmul(
                                ps4[qc][:],
                                wq_rows[f][:, e],
                                xt_t[:, e, qc * 512 : (qc + 1) * 512],
                                start=(e == 0),
                                stop=(e == ET - 1),
                            )
                    for qc in range(NKC):
                        nc.scalar.add(
                            qt_t[:, f, qc * 512 : (qc + 1) * 512],
                            ps4[qc][:],
                            bq_t[:, f : f + 1],
                        )

                # stage own half to DRAM, pair-allgather into global order,
                # read back both halves; overlaps with the V projection
                nc.gpsimd.dma_start(qh_d[:], qt_t[:])
                nc.gpsimd.collective_compute(
                    "AllGather",
                    mybir.AluOpType.bypass,
                    replica_groups=groups,
                    ins=[qh_d[:]],
                    outs=[qg_d[:]],
                )
                for g in range(2):
                    nc.gpsimd.dma_start(
                        qt_g[:, :, g * SK : (g + 1) * SK], qg_d[g]
                    )

                # ---- V projection: v[k, f] = sum_e xt[e, k] * wv[e, f] ----
                for kt in range(KT):
                    ps2 = [
                        psum.tile([P, 512], f32, tag="mm", name=f"v{kt}_{fc}")
                        for fc in range(NFC)
                    ]
                    for e in range(ET):
                        for fc in range(NFC):
                            nc.tensor.matmul(
                                ps2[fc][:],
                                xt_t[:, e, kt * P : (kt + 1) * P],
                                wv_t[:, e, fc * 512 : (fc + 1) * 512],
                                start=(e == 0),
                                stop=(e == ET - 1),
                            )
                    for fc in range(NFC):
                        nc.vector.tensor_copy(
                            v_t[:, kt, fc * 512 : (fc + 1) * 512], ps2[fc][:]
                        )

            with tc.tile_pool(name="ptp", bufs=1) as ptp:
                pt_t = ptp.tile([P, KT, S], bf16, tag="pt")

                # ---- scores^T (fp8 DoubleRow) + exp; rowsums in a second
                # pass so the PE never waits on the scalar exp ----
                DR = mybir.MatmulPerfMode.DoubleRow
                scale = float(1.0 / np.sqrt(np.float32(E)))
                for qh in range(2):
                    for kt in range(KT):
                        ps2 = [
                            psum.tile([P, 512], f32, tag="mm", name=f"s{kt}_{qc}")
                            for qc in range(2)
                        ]
                        for fp in range(ET // 2):
                            for qc in range(2):
                                col = qh * 1024 + qc * 512
                                nc.tensor.matmul(
                                    ps2[qc][:],
                                    kt_t[:, 2 * fp : 2 * fp + 2, kt * P : (kt + 1) * P],
                                    qt_g[:, 2 * fp : 2 * fp + 2, col : col + 512],
                                    start=(fp == 0),
                                    stop=(fp == ET // 2 - 1),
                                    perf_mode=DR,
                                )
                        for qc in range(2):
                            col = qh * 1024 + qc * 512
                            nc.scalar.activation(
                                pt_t[:, kt, col : col + 512], ps2[qc][:], ACT.Exp,
                                scale=scale,
                            )
                    for kt in range(KT):
                        for qc in range(2):
                            col = qh * 1024 + qc * 512
                            nc.tensor.matmul(
                                rs_ps[qc][:],
                                ones_t[:, 0:1],
                                pt_t[:, kt, col : col + 512],
                                start=(kt == 0),
                                stop=(kt == KT - 1),
                            )
                    for qc in range(2):
                        col = qh * 1024 + qc * 512
                        nc.vector.tensor_copy(
                            rs_sb[:, col : col + 512], rs_ps[qc][:]
                        )

                # ---- O = pt^T @ v, unnormalized; store bf16 ----
                for qt in range(QT):
                    po = [
                        psum.tile([P, 512], f32, tag="mm", name=f"o{qt}_{fc}")
                        for fc in range(NFC)
                    ]
                    for kt in range(KT):
                        for fc in range(NFC):
                            nc.tensor.matmul(
                                po[fc][:],
                                pt_t[:, kt, qt * P : (qt + 1) * P],
                                v_t[:, kt, fc * 512 : (fc + 1) * 512],
                                start=(kt == 0),
                                stop=(kt == KT - 1),
                            )
                    o_sb = obp.tile([P, E], bf16, tag="ob")
                    for fc in range(NFC):
                        nc.vector.tensor_copy(
                            o_sb[:, fc * 512 : (fc + 1) * 512], po[fc][:]
                        )
                    nc.gpsimd.dma_start(ou[qt * P : (qt + 1) * P, :], o_sb[:])
                nc.sync.dma_start(rs[:], rs_sb[:])


_NC_CACHE = {}


def build_nc(E=1024, S=2048, SK=1024):
    key = (E, S, SK)
    if key in _NC_CACHE:
        return _NC_CACHE[key]
    import concourse.bacc as bacc

    nc = bacc.Bacc(None, target_bir_lowering=False)
    _emit(nc, E=E, S=S, SK=SK)
    nc.finalize()
    _NC_CACHE[key] = nc
    return nc


def _round_f32r(a):
    """Round fp32 to fp32r (tf32-like: 11 explicit mantissa bits, RNE)."""
    u = np.ascontiguousarray(a, np.float32).view(np.uint32)
    u = u + np.uint32(0x7FF) + ((u >> np.uint32(12)) & np.uint32(1))
    return (u & np.uint32(0xFFFFF000)).view(np.float32)


def make_in_maps(x, Wq, bq, Wk, bk, Wv, bv, E=1024, S=2048, SK=1024):
    """Host-side prep: per-core input dicts for run_bass_kernel_spmd."""
    import ml_dtypes

    bf16 = ml_dtypes.bfloat16
    ET = E // P
    scale = np.float32(1.0 / np.sqrt(np.float32(E)))
    x = np.asarray(x, np.float32)
    B = x.shape[0]
    n_half = S // SK

    def wtile(w):  # [f_tile, p(e), e_tile, c(f)] stationary blocks
        return np.ascontiguousarray(
            np.asarray(w, np.float32).reshape(ET, P, ET, P).transpose(0, 3, 2, 1)
        ).astype(bf16)

    wq8 = wtile(Wq)
    wk8 = wtile(Wk)
    # wv8[e, p, f] = Wv[f, e*128+p]
    wv8 = np.ascontiguousarray(
        np.asarray(Wv, np.float32).T.reshape(ET, P, E)
    ).astype(bf16)
    bq8 = np.ascontiguousarray(np.asarray(bq, np.float32).reshape(ET, P).T)
    bk8 = np.ascontiguousarray(np.asarray(bk, np.float32).reshape(ET, P).T)
    ones8 = np.ones((P, 512), bf16)

    in_maps = []
    for c in range(B * n_half):
        b, h = divmod(c, n_half)
        xt_full = x[b].T  # [E, S]
        if h == 1:
            xt_full = np.concatenate([xt_full[:, SK:], xt_full[:, :SK]], axis=1)
        xt8 = np.ascontiguousarray(xt_full.reshape(ET, P, S)).astype(bf16)
        in_maps.append(
            {
                "xt8": xt8,
                "wq8": wq8,
                "wk8": wk8,
                "wv8": wv8,
                "bq8": bq8,
                "bk8": bk8,
                "ones8": ones8,
            }
        )
    return in_maps


def kernel(x, Wq, bq, Wk, bk, Wv, bv):
    from concourse.bass_utils import run_bass_kernel_spmd

    E, S, SK = 1024, 2048, 1024
    x = np.asarray(x, np.float32)
    B = x.shape[0]
    n_half = S // SK
    nc = build_nc(E=E, S=S, SK=SK)
    in_maps = make_in_maps(x, Wq, bq, Wk, bk, Wv, bv, E=E, S=S, SK=SK)
    n_cores = len(in_maps)
    res = run_bass_kernel_spmd(nc, in_maps, list(range(n_cores)))

    bvf = np.asarray(bv, np.float32)
    out = np.empty((B, S, E), np.float32)
    for b in range(B):
        osum = None
        rsum = None
        for h in range(n_half):
            r = res.results[b * n_half + h]
            o_h = np.asarray(r["ou"]).astype(np.float32)
            rs_h = np.asarray(r["rs"]).astype(np.float32).reshape(S)
            osum = o_h if osum is None else osum + o_h
            rsum = rs_h if rsum is None else rsum + rs_h
        out[b] = osum / rsum[:, None] + bvf[None, :]
    return out


# revision 11
# speedup vs baseline: 1.5248x; 1.0306x over previous
"""Single-head attention (B=4, S=2048, E=1024, fp32) on 8 trn2 NeuronCores.

Sharding: (batch, key-half) -> 8 shards. Core c handles batch c//2 and the
key/value rows [h*1024, (h+1)*1024) with h = c%2. Each core computes the Q
projection for ALL 2048 queries of its batch, K/V projections for its own
1024 keys, exp(scores^T) against those keys, the unnormalized partial output
O_h = exp(S^T)^T @ V_h and the partial softmax denominators rs_h. The host
combines: out = (O_0 + O_1) / (rs_0 + rs_1) + bv  (the V bias commutes with
the softmax average, so it is added once on the host).

Dtype split: STATIONARY matmul operands are fp32r (standard 2-XBUS
LDWEIGHTS hides behind the moving stream; bf16 FWL grabs all 4 XBUSes and
serializes ~45ns/matmul), MOVING operands are bf16 (halves SBUF + DMA).
Exception: the V projection's stationary is the bf16 xt tile (cheaper than
keeping a second fp32r copy of x).

  xt [128, 8e, 2048] bf16  x[b]^T, key-half columns first (host permute).
  w  [128, 8e, 128] f32r   Wq^T*scale / Wk^T stationary tiles (streamed).
  qt [128, 8f, 2048] bf16  Q^T - moving operand of scores.
  kt [128, 8f, 1024] f32r  K^T - stationary of scores.
  wv [128, 8e, 1024] bf16  Wv^T - moving operand of the V projection.
  v  [128, 8k, 1024] bf16  V - moving operand of O.
  pt [128, 8k, 2048] f32r  exp(S^T) - stationary of O, moving of rowsum.

Rowsums come from ones^T @ exp tiles on the PE. A burst of tiny warmup
matmuls runs during the initial input DMA so the PE's activity-based clock
ramp (1.2 -> 2.4 GHz) completes before the first real matmul.
"""

import numpy as np

P = 128


def _emit(nc, E=1024, S=2048, SK=1024):
    import concourse.mybir as mybir
    import concourse.tile as tile

    f32 = mybir.dt.float32
    f32r = mybir.dt.float32r
    bf16 = mybir.dt.bfloat16
    fp8 = mybir.dt.float8e4
    ACT = mybir.ActivationFunctionType

    ET = E // P     # e/f tiles (8)
    QT = S // P     # q tiles (16)
    KT = SK // P    # k tiles (8)
    NQC = S // 512  # q chunks (4)
    NKC = SK // 512  # k chunks (2)
    NFC = E // 512  # f chunks (2)

    xt8 = nc.dram_tensor("xt8", [ET, P, S], bf16, kind="ExternalInput")
    wq8 = nc.dram_tensor("wq8", [ET, P, ET, P], bf16, kind="ExternalInput")
    wk8 = nc.dram_tensor("wk8", [ET, P, ET, P], bf16, kind="ExternalInput")
    wv8 = nc.dram_tensor("wv8", [ET, P, E], bf16, kind="ExternalInput")
    bq8 = nc.dram_tensor("bq8", [P, ET], f32, kind="ExternalInput")
    bk8 = nc.dram_tensor("bk8", [P, ET], f32, kind="ExternalInput")
    ones8 = nc.dram_tensor("ones8", [P, 512], bf16, kind="ExternalInput")
    ou = nc.dram_tensor("ou", [S, E], bf16, kind="ExternalOutput")
    rs = nc.dram_tensor("rs", [1, S], f32, kind="ExternalOutput")

    groups = [[2 * i, 2 * i + 1] for i in range(4)]

    with tile.TileContext(nc) as tc:
        with (
            tc.tile_pool(name="dramp", bufs=1, space="DRAM") as dramp,
            tc.tile_pool(name="psum", bufs=6, space="PSUM") as psum,
            tc.tile_pool(name="small", bufs=1) as small,
            tc.tile_pool(name="persist", bufs=1) as pers,
            tc.tile_pool(name="obuf", bufs=3) as obp,
        ):
            ones_t = small.tile([P, 512], bf16, tag="ones")
            nc.gpsimd.dma_start(ones_t[:], ones8[:])
            bq_t = small.tile([P, ET], f32, tag="bq")
            nc.sync.dma_start(bq_t[:], bq8[:])
            bk_t = small.tile([P, ET], f32, tag="bk")
            nc.sync.dma_start(bk_t[:], bk8[:])
            rs_sb = small.tile([1, S], f32, tag="rssb")

            qt_g = pers.tile([P, ET, S], fp8, tag="qtg")
            kt_t = pers.tile([P, ET, SK], fp8, tag="kt")
            v_t = pers.tile([P, KT, E], bf16, tag="v")

            rs_ps = [
                psum.tile([1, 512], f32, tag=f"rs{qc}", name=f"rs{qc}", bufs=1)
                for qc in range(2)
            ]

            with (
                tc.tile_pool(name="ph1", bufs=1) as ph1,
                tc.tile_pool(name="wstream", bufs=3) as wsp,
            ):
                xt_t = ph1.tile([P, ET, S], bf16, tag="xt")
                wv_t = ph1.tile([P, ET, E], bf16, tag="wv")
                qt_t = ph1.tile([P, ET, SK], fp8, tag="qt")
                qh_d = dramp.tile([P, ET, SK], fp8, tag="qhd")
                qg_d = dramp.tile([2, P, ET, SK], fp8, tag="qgd")

                # ---- input DMA: sync ring = weights; xt alternates over the
                # gpsimd + vector rings, key-half columns first ----
                wq_rows = []
                w_t = wsp.tile([P, ET, P], bf16, tag="w", name="wq_f0")
                nc.sync.dma_start(w_t[:], wq8[0])
                wq_rows.append(w_t)
                xt_rings = [nc.gpsimd, nc.scalar]
                for kc in range(NKC):  # key half, chunk-sized for early starts
                    for e in range(ET):
                        xt_rings[e % 2].dma_start(
                            xt_t[:, e, kc * 512 : (kc + 1) * 512],
                            xt8[e, :, kc * 512 : (kc + 1) * 512],
                        )
                for f in range(1, ET):
                    w_t = wsp.tile([P, ET, P], bf16, tag="w", name=f"wq_f{f}")
                    nc.sync.dma_start(w_t[:], wq8[f])
                    wq_rows.append(w_t)
                for e in range(ET):  # q-only columns, one wide DMA each
                    xt_rings[e % 2].dma_start(xt_t[:, e, SK:S], xt8[e, :, SK:S])
                wk_rows = []
                for f in range(ET):
                    w_t = wsp.tile([P, ET, P], bf16, tag="w", name=f"wk_f{f}")
                    nc.sync.dma_start(w_t[:], wk8[f])
                    wk_rows.append(w_t)
                for e in range(ET):
                    nc.scalar.dma_start(wv_t[:, e], wv8[e])

                # ---- PE warmup (HAM clock ramp) while input DMAs stream:
                # full-width matmuls span ~6us, bridging to first data ----
                for i in range(14):
                    nc.tensor.matmul(
                        rs_ps[0][:], ones_t[:, 0:1], ones_t[:],
                        start=True, stop=True,
                    )

                # ---- Q projection, own query half only (the first SK
                # permuted columns = global queries [h*SK,(h+1)*SK)) ----
                for f in range(ET):
                    ps4 = [
                        psum.tile([P, 512], f32, tag="mm", name=f"q{f}_{qc}")
                        for qc in range(NKC)
                    ]
                    for e in range(ET):
                        for qc in range(NKC):
                            nc.tensor.matmul(
                                ps4[qc][:],
                                wq_rows[f][:, e],
                                xt_t[:, e, qc * 512 : (qc + 1) * 512],
                                start=(e == 0),
                                stop=(e == ET - 1),
                            )
                    for qc in range(NKC):
                        nc.scalar.add(
                            qt_t[:, f, qc * 512 : (qc + 1) * 512],
                            ps4[qc][:],
                            bq_t[:, f : f + 1],
                        )

                # stage own half to DRAM, pair-allgather into global order,
                # read back both halves; overlaps with the V projection
                nc.gpsimd.dma_start(qh_d[:], qt_t[:])
                nc.gpsimd.collective_compute(
                    "AllGather",
                    mybir.AluOpType.bypass,
                    replica_groups=groups,
                    ins=[qh_d[:]],
                    outs=[qg_d[:]],
                )
                for g in range(2):
                    nc.gpsimd.dma_start(
                        qt_g[:, :, g * SK : (g + 1) * SK], qg_d[g]
                    )

                # ---- K projection (key half = first SK columns of xt) ----
                for f in range(ET):
                    ps2 = [
                        psum.tile([P, 512], f32, tag="mm", name=f"k{f}_{kc}")
                        for kc in range(NKC)
                    ]
                    for e in range(ET):
                        for kc in range(NKC):
                            nc.tensor.matmul(
                                ps2[kc][:],
                                wk_rows[f][:, e],
                                xt_t[:, e, kc * 512 : (kc + 1) * 512],
                                start=(e == 0),
                                stop=(e == ET - 1),
                            )
                    for kc in range(NKC):
                        nc.scalar.add(
                            kt_t[:, f, kc * 512 : (kc + 1) * 512],
                            ps2[kc][:],
                            bk_t[:, f : f + 1],
                        )

                # ---- V projection: v[k, f] = sum_e xt[e, k] * wv[e, f] ----
                for kt in range(KT):
                    ps2 = [
                        psum.tile([P, 512], f32, tag="mm", name=f"v{kt}_{fc}")
                        for fc in range(NFC)
                    ]
                    for e in range(ET):
                        for fc in range(NFC):
                            nc.tensor.matmul(
                                ps2[fc][:],
                                xt_t[:, e, kt * P : (kt + 1) * P],
                                wv_t[:, e, fc * 512 : (fc + 1) * 512],
                                start=(e == 0),
                                stop=(e == ET - 1),
                            )
                    for fc in range(NFC):
                        nc.vector.tensor_copy(
                            v_t[:, kt, fc * 512 : (fc + 1) * 512], ps2[fc][:]
                        )

            with tc.tile_pool(name="ptp", bufs=1) as ptp:
                pt_t = ptp.tile([P, KT, S], bf16, tag="pt")

                # ---- scores^T (fp8 DoubleRow) + exp; rowsums in a second
                # pass so the PE never waits on the scalar exp ----
                DR = mybir.MatmulPerfMode.DoubleRow
                scale = float(1.0 / np.sqrt(np.float32(E)))
                for qh in range(2):
                    for kt in range(KT):
                        ps2 = [
                            psum.tile([P, 512], f32, tag="mm", name=f"s{kt}_{qc}")
                            for qc in range(2)
                        ]
                        for fp in range(ET // 2):
                            for qc in range(2):
                                col = qh * 1024 + qc * 512
                                nc.tensor.matmul(
                                    ps2[qc][:],
                                    kt_t[:, 2 * fp : 2 * fp + 2, kt * P : (kt + 1) * P],
                                    qt_g[:, 2 * fp : 2 * fp + 2, col : col + 512],
                                    start=(fp == 0),
                                    stop=(fp == ET // 2 - 1),
                                    perf_mode=DR,
                                )
                        for qc in range(2):
                            col = qh * 1024 + qc * 512
                            nc.scalar.activation(
                                pt_t[:, kt, col : col + 512], ps2[qc][:], ACT.Exp,
                                scale=scale,
                            )
                    for kt in range(KT):
                        for qc in range(2):
                            col = qh * 1024 + qc * 512
                            nc.tensor.matmul(
                                rs_ps[qc][:],
                                ones_t[:, 0:1],
                                pt_t[:, kt, col : col + 512],
                                start=(kt == 0),
                                stop=(kt == KT - 1),
                            )
                    for qc in range(2):
                        col = qh * 1024 + qc * 512
                        nc.vector.tensor_copy(
                            rs_sb[:, col : col + 512], rs_ps[qc][:]
                        )

                # ---- O = pt^T @ v, unnormalized; store bf16 ----
                for qt in range(QT):
                    po = [
                        psum.tile([P, 512], f32, tag="mm", name=f"o{qt}_{fc}")
                        for fc in range(NFC)
                    ]
                    for kt in range(KT):
                        for fc in range(NFC):
                            nc.tensor.matmul(
                                po[fc][:],
                                pt_t[:, kt, qt * P : (qt + 1) * P],
                                v_t[:, kt, fc * 512 : (fc + 1) * 512],
                                start=(kt == 0),
                                stop=(kt == KT - 1),
                            )
                    o_sb = obp.tile([P, E], bf16, tag="ob")
                    for fc in range(NFC):
                        nc.vector.tensor_copy(
                            o_sb[:, fc * 512 : (fc + 1) * 512], po[fc][:]
                        )
                    nc.gpsimd.dma_start(ou[qt * P : (qt + 1) * P, :], o_sb[:])
                nc.sync.dma_start(rs[:], rs_sb[:])


_NC_CACHE = {}


def build_nc(E=1024, S=2048, SK=1024):
    key = (E, S, SK)
    if key in _NC_CACHE:
        return _NC_CACHE[key]
    import concourse.bacc as bacc

    nc = bacc.Bacc(None, target_bir_lowering=False)
    _emit(nc, E=E, S=S, SK=SK)
    nc.finalize()
    _NC_CACHE[key] = nc
    return nc


def _round_f32r(a):
    """Round fp32 to fp32r (tf32-like: 11 explicit mantissa bits, RNE)."""
    u = np.ascontiguousarray(a, np.float32).view(np.uint32)
    u = u + np.uint32(0x7FF) + ((u >> np.uint32(12)) & np.uint32(1))
    return (u & np.uint32(0xFFFFF000)).view(np.float32)


def make_in_maps(x, Wq, bq, Wk, bk, Wv, bv, E=1024, S=2048, SK=1024):
    """Host-side prep: per-core input dicts for run_bass_kernel_spmd."""
    import ml_dtypes

    bf16 = ml_dtypes.bfloat16
    ET = E // P
    scale = np.float32(1.0 / np.sqrt(np.float32(E)))
    x = np.asarray(x, np.float32)
    B = x.shape[0]
    n_half = S // SK

    def wtile(w):  # [f_tile, p(e), e_tile, c(f)] stationary blocks
        return np.ascontiguousarray(
            np.asarray(w, np.float32).reshape(ET, P, ET, P).transpose(0, 3, 2, 1)
        ).astype(bf16)

    wq8 = wtile(Wq)
    wk8 = wtile(Wk)
    # wv8[e, p, f] = Wv[f, e*128+p]
    wv8 = np.ascontiguousarray(
        np.asarray(Wv, np.float32).T.reshape(ET, P, E)
    ).astype(bf16)
    bq8 = np.ascontiguousarray(np.asarray(bq, np.float32).reshape(ET, P).T)
    bk8 = np.ascontiguousarray(np.asarray(bk, np.float32).reshape(ET, P).T)
    ones8 = np.ones((P, 512), bf16)

    in_maps = []
    for c in range(B * n_half):
        b, h = divmod(c, n_half)
        xt_full = x[b].T  # [E, S]
        if h == 1:
            xt_full = np.concatenate([xt_full[:, SK:], xt_full[:, :SK]], axis=1)
        xt8 = np.ascontiguousarray(xt_full.reshape(ET, P, S)).astype(bf16)
        in_maps.append(
            {
                "xt8": xt8,
                "wq8": wq8,
                "wk8": wk8,
                "wv8": wv8,
                "bq8": bq8,
                "bk8": bk8,
                "ones8": ones8,
            }
        )
    return in_maps


def kernel(x, Wq, bq, Wk, bk, Wv, bv):
    from concourse.bass_utils import run_bass_kernel_spmd

    E, S, SK = 1024, 2048, 1024
    x = np.asarray(x, np.float32)
    B = x.shape[0]
    n_half = S // SK
    nc = build_nc(E=E, S=S, SK=SK)
    in_maps = make_in_maps(x, Wq, bq, Wk, bk, Wv, bv, E=E, S=S, SK=SK)
    n_cores = len(in_maps)
    res = run_bass_kernel_spmd(nc, in_maps, list(range(n_cores)))

    bvf = np.asarray(bv, np.float32)
    out = np.empty((B, S, E), np.float32)
    for b in range(B):
        osum = None
        rsum = None
        for h in range(n_half):
            r = res.results[b * n_half + h]
            o_h = np.asarray(r["ou"]).astype(np.float32)
            rs_h = np.asarray(r["rs"]).astype(np.float32).reshape(S)
            osum = o_h if osum is None else osum + o_h
            rsum = rs_h if rsum is None else rsum + rs_h
        out[b] = osum / rsum[:, None] + bvf[None, :]
    return out
